# revision 28
# baseline (speedup 1.0000x reference)
"""Trainium2 Bass kernel for nn_AdvancedSpikingChatModel.

Model: spike-encode embeddings -> 6 spiking-transformer blocks (LIF gates +
decaying linear-attention recurrence over T=16) -> LIF output head with
spike-count accumulation over V=32000 vocab.

Strategy (8 NeuronCores, SPMD, two launches):
  Launch 1 (blocks): data-parallel over the 256 folded (b,s) rows, 32/core.
    Features on partitions, (t, row) on the free dim; weights stationary.
  Launch 2 (head): vocab-parallel, 4096 padded cols/core, all 256 rows.

Precision: matmuls run as fp16 hi/lo split passes (x@W = xh@Wh + xl@Wh +
xh@Wl accumulated in fp32 PSUM; dropped xl@Wl term ~2^-22) — fp32-grade
results at the PE's fp16 rate (fp32 matmuls cost ~2.6x on TRN2). The LIF
threshold compare (v >= 1) makes anything coarser (bf16/fp32r) flip spikes.
Spike matrices (0/1) are exact in fp16, so spike-side matmuls use 2 passes.

LIF decay 0.5 folded into weights: w' = 0.5*(min(w,1) - (w>=1)) + a, emitted
as ONE custom DVE op per step; spikes s = (w >= 1) recovered in one batched
GPSIMD pass per scan; spike counts via add-tree (GPSIMD + DVE).
"""

import numpy as np

import concourse.mybir as mybir
import concourse.tile as tile
from concourse import bacc
from concourse.bass_utils import run_bass_kernel_spmd

F32 = mybir.dt.float32
F16 = mybir.dt.float16
F8E4 = mybir.dt.float8e4
OP = mybir.AluOpType
AF = mybir.ActivationFunctionType
DR = mybir.MatmulPerfMode.DoubleRow

B, S, D, T, L, F, V = 2, 128, 256, 16, 6, 1024, 32000
N = B * S
NCORE = 8
R = N // NCORE       # 32 rows/core in launch 1
TR = T * R           # 512
KC = D // 128
FC = F // 128
VPAD = 32768
VSH = VPAD // NCORE  # 4096
VCH = VSH // 128     # 32 chunks
TN = T * N           # 4096
EPS = 1e-5

# fp16 weight slab offsets (fp16 words per partition, per layer).
# Gates/W1 hi tiles are pre-scaled by 2^14 (PSUM scale shared with the
# fp8 DoubleRow correction passes; drained with ACT scale 2^-14).
GH_OFF = 0
WOH_OFF = GH_OFF + 12 * 128
WOL_OFF = WOH_OFF + 4 * 128
W1H_OFF = WOL_OFF + 4 * 128
W2H_OFF = W1H_OFF + 16 * 128
W2L_OFF = W2H_OFF + 16 * 128
W16 = W2L_OFF + 16 * 128
# fp8 slab: [L, 128, 2(B/C), 14 banks, 2 kc, 128] — banks 0-5 gates, 6-13 W1.
# B = e4m3(Wh * 2^3) pairs with x-lo * 2^11; C = e4m3(Wl * 2^14) with x-hi.
NB8 = 14
# fp32 smalls: b1(8) b2(2) ln(8)
B1_OFF = 0
B2_OFF = 8
LN_OFF = 10
WS = 18

_LIF_OP = None
_LIFCNT_OP = None


def _register_op(name, spec):
    from concourse.dve_ops import (
        DveOp, OPS, _SUB_OPCODE_FOR_NAME, CUSTOM_DVE_SPECS)
    from concourse.dve_spec import lower
    from concourse.dve_uop import DveOpSpec

    if name in _SUB_OPCODE_FOR_NAME:
        return next(o for o in OPS if o.name == name)
    op = DveOp(name, spec, subdim=False, uops_sha={})
    row = 1 + len(OPS)
    OPS.append(op)
    _SUB_OPCODE_FOR_NAME[name] = row
    CUSTOM_DVE_SPECS[name] = spec
    for ver in ("v3",):
        s = DveOpSpec(name=name, opcode=row, uops=lower(spec, ver=ver),
                      rd1_en=True)
        op.uops_sha[ver] = s.sha(ver)
    return op


def _get_lif_op():
    """LIF step as a custom DVE op: out = (min(w,1) - (w>=1))*0.5 + a."""
    global _LIF_OP
    if _LIF_OP is None:
        from concourse.dve_spec import Spec, Src0, Src1, C0, One, minn
        body = (minn(Src0, One) - (Src0 >= One)) * C0 + Src1
        _LIF_OP = _register_op("LIF_STEP_ANT", Spec(
            body=body,
            reference=lambda in0, in1, s0, s1, imm2:
                (np.minimum(in0, 1.0) - (in0 >= 1.0)) * s0 + in1,
        ))
    return _LIF_OP


def _get_lifcnt_op():
    """Fused LIF step + spike count, one DVE pass (8 ALU stages).

    State U = V + A/2 in one fp32: V = w/16 in (-0.25, 0.25) is the membrane,
    A the spike count. s0 = 0.25 (latches derive threshold 1/16 = s0^2 and
    decay 0.5 = s0+s0), s1 = 1.5*2^22 (magic: (U+M)-M rounds U to the
    nearest 0.5 multiple = A/2, exact for |V| < 0.25 both signs).
    Spike branch outputs One, halved to +0.5 == one count unit, V reset 0.
    in1 = a/16 (pre-scaled in the PSUM drain)."""
    global _LIFCNT_OP
    if _LIFCNT_OP is None:
        from concourse.dve_spec import (
            Spec, Src0, Src1, C0, C1, One, Latch, select)
        T16 = Latch(C0 * C0)
        Half = Latch(C0 + C0)
        m1 = Src0 + C1
        r = m1 - C1
        V = Src0 - r
        g = V >= T16
        body = select(g, One, V) * Half + (Src1 + r)

        def ref(in0, in1, s0, s1, imm2):
            f32 = np.float32
            in0 = np.asarray(in0, f32)
            in1 = np.asarray(in1, f32)
            r = (in0 + f32(s1)).astype(f32) - f32(s1)
            V = in0 - r
            g = V >= f32(s0) * f32(s0)
            selv = np.where(g, f32(1.0), V)
            return selv * (f32(s0) + f32(s0)) + (in1 + r)

        _LIFCNT_OP = _register_op("LIF_CNT_ANT", Spec(body=body, reference=ref))
    return _LIFCNT_OP


def _sigmoid(x):
    return 1.0 / (1.0 + np.exp(-x))


def _encode_spikes(input_ids, token_embedding, pos_embedding, noise, unif):
    """Host-side rate coding; (0.7*rate + 0.3*temp > 0.5) == rate exactly."""
    emb = token_embedding[input_ids] + pos_embedding[None, :S]
    p = np.clip(_sigmoid(emb) * 0.8 + 0.1 + noise * 0.05, 0.0, 1.0)
    return (unif < p[None]).astype(np.float32)


def _split16(x):
    hi = x.astype(np.float16)
    lo = (x - hi.astype(np.float32)).astype(np.float16)
    return hi, lo


def _mm16(nc, ps, passes, dst_ap, bias=0.0, name="mmb", free=512, scale=1.0):
    """Accumulate matmul passes into one PSUM bank, ACT-copy(+bias) out.
    A pass is (lhsT, rhs) fp16 or (lhsT, rhs, perf_mode) for fp8 DR."""
    bank = ps.tile([128, free], F32, tag="mm", name=name, bufs=4)
    npass = len(passes)
    for i, p in enumerate(passes):
        pm = p[2] if len(p) > 2 else None
        nc.tensor.matmul(bank[:], p[0], p[1],
                         start=(i == 0), stop=(i == npass - 1), perf_mode=pm)
    nc.scalar.activation(dst_ap, bank[:], AF.Identity, bias=bias, scale=scale)


def _w_scan(nc, lif, w_buf, z0, a_fn, nt=T, sliced=False):
    """w_t = (min(w_{t-1},1) - (w_{t-1}>=1))*0.5 + a_t via the custom op.
    in1 must keep >=2 free dims (STT encoding; the TTSS form runs ~10x slower)."""
    for t in range(nt):
        if sliced:
            out = w_buf[:, t:t + 1, :]
            in0 = z0[:] if t == 0 else w_buf[:, t - 1:t, :]
        else:
            out = w_buf[:, t]
            in0 = z0[:] if t == 0 else w_buf[:, t - 1]
        nc.vector._custom_dve(lif, out=out, in0=in0, in1=a_fn(t), s0=0.5)


def _layer_norm(nc, ps, sb, u, sq_buf, gamma_col, beta_col, out_fn,
                ones_col, ones_row, eps_col, csl, W, identity=False):
    """LN over features (partitions x KC chunks) on a column slice csl of
    width W. u: [128, KC, TR] fp32; out_fn(kc) -> dst AP for that slice.
    identity=True skips the gamma/beta affine (gamma==1, beta==0)."""
    for kc in range(KC):
        nc.scalar.activation(sq_buf[:, kc, csl], u[:, kc, csl], AF.Square)
    ps_m = ps.tile([1, W], F32, tag="st", name="ps_m", bufs=2)
    ps_q = ps.tile([1, W], F32, tag="st", name="ps_q", bufs=2)
    for kc in range(KC):
        nc.tensor.matmul(ps_m[:], ones_col[:], u[:, kc, csl],
                         start=(kc == 0), stop=(kc == KC - 1))
    for kc in range(KC):
        nc.tensor.matmul(ps_q[:], ones_col[:], sq_buf[:, kc, csl],
                         start=(kc == 0), stop=(kc == KC - 1))
    m_sb = sb.tile([1, W], F32, name="m_sb", tag="m_sb", bufs=2)
    q_sb = sb.tile([1, W], F32, name="q_sb", tag="q_sb", bufs=2)
    nc.scalar.mul(m_sb[:], ps_m[:], 1.0 / D)
    nc.scalar.mul(q_sb[:], ps_q[:], 1.0 / D)
    ve = sb.tile([1, W], F32, name="ve", tag="ve", bufs=2)
    nc.vector.tensor_mul(out=ve[:], in0=m_sb[:], in1=m_sb[:])
    nc.vector.tensor_sub(out=ve[:], in0=q_sb[:], in1=ve[:])
    # rstd = 1/sqrt(var+eps): ACT sqrt (eps via bias) + fast reciprocal
    r0 = sb.tile([1, W], F32, name="r0", tag="r0", bufs=2)
    nc.scalar.activation(r0[:], ve[:], AF.Sqrt, bias=eps_col[:])
    nc.vector.reciprocal_approx_fast(r0[:], r0[:])
    pb_m = ps.tile([128, W], F32, tag="bc", name="pb_m", bufs=2)
    pb_r = ps.tile([128, W], F32, tag="bc", name="pb_r", bufs=2)
    nc.tensor.matmul(pb_m[:], ones_row[:], m_sb[:], start=True, stop=True)
    nc.tensor.matmul(pb_r[:], ones_row[:], r0[:], start=True, stop=True)
    for kc in range(KC):
        o = out_fn(kc)
        nc.vector.tensor_sub(out=o, in0=u[:, kc, csl], in1=pb_m[:])
        nc.vector.tensor_mul(out=o, in0=o, in1=pb_r[:])
        if not identity:
            nc.vector.tensor_scalar(out=o, in0=o, scalar1=gamma_col(kc),
                                    scalar2=beta_col(kc), op0=OP.mult,
                                    op1=OP.add)


def build_blocks(ln_id=True):
    lif = _get_lif_op()
    nc = bacc.Bacc("TRN2", target_bir_lowering=False)
    x0_d = nc.dram_tensor("x0", [128, KC, TR], F32, kind="ExternalInput")
    w16_d = nc.dram_tensor("w16", [L, 128, W16], F16, kind="ExternalInput")
    w8_d = nc.dram_tensor("w8", [L, 128, 2, NB8, 2, 128], F8E4,
                          kind="ExternalInput")
    w32_d = nc.dram_tensor("w32", [L, 128, WS], F32, kind="ExternalInput")
    h_d = nc.dram_tensor("h_out", [128, KC, TR], F32, kind="ExternalOutput")

    with tile.TileContext(nc) as tc:
        with tc.tile_pool(name="wp", bufs=2) as wp, \
             tc.tile_pool(name="sb", bufs=1) as sb, \
             tc.tile_pool(name="ps", bufs=1, space="PSUM") as ps:

            ones_col = sb.tile([128, 1], F32)
            ones_row = sb.tile([1, 128], F32)
            eps_col = sb.tile([1, 1], F32)
            nc.vector.memset(ones_col[:], 1.0)
            nc.vector.memset(ones_row[:], 1.0)
            nc.vector.memset(eps_col[:], EPS)

            x_cur = sb.tile([128, KC, TR], F32)
            nc.sync.dma_start(x_cur[:], x0_d.ap()[:])

            xh = sb.tile([128, KC, TR], F16)
            xl = sb.tile([128, KC, TR], F16)
            aga = sb.tile([128, 6, TR // 2], F32)
            agb = sb.tile([128, 6, TR // 2], F32)
            wg_buf = sb.tile([128, T, 6, R], F32)
            s_buf = sb.tile([128, T, 6, R], F16)
            kv_buf = sb.tile([128, T, KC, R], F16)
            h_buf = sb.tile([128, T, KC, R], F32)
            hh16 = sb.tile([128, T, KC, R], F16)
            hl16 = sb.tile([128, T, KC, R], F16)
            rhh = sb.tile([128, T, KC, R], F16)
            rhl = sb.tile([128, T, KC, R], F16)
            at_buf = sb.tile([128, KC, TR], F32)
            u_buf = sb.tile([128, KC, TR], F32)
            sq_buf = sb.tile([128, KC, TR], F32)
            x1_buf = sb.tile([128, KC, TR], F32)
            x1h = sb.tile([128, KC, TR], F16)
            x1l = sb.tile([128, KC, TR], F16)
            a1a = sb.tile([128, FC, TR // 2], F32)
            a1b = sb.tile([128, FC, TR // 2], F32)
            w1_buf = sb.tile([128, T, FC, R], F32)
            s1_buf = sb.tile([128, T, FC, R], F16)
            a2a = sb.tile([128, KC, TR // 2], F32)
            a2b = sb.tile([128, KC, TR // 2], F32)
            w2_buf = sb.tile([128, T, KC, R], F32)
            s2_buf = sb.tile([128, T, KC, R], F16)
            zg = sb.tile([128, 6, R], F32)
            zh = sb.tile([128, KC, R], F32)
            z1 = sb.tile([128, FC, R], F32)
            nc.vector.memset(zg[:], 0.0)
            nc.vector.memset(zh[:], 0.0)
            nc.vector.memset(z1[:], 0.0)

            wl16 = [wp.tile([128, W16], F16, tag="w16", name=f"w16_{i}")
                    for i in range(L)]
            wl8 = [wp.tile([128, 2, NB8, 2, 128], F8E4, tag="w8",
                           name=f"w8_{i}") for i in range(L)]
            wl32 = [wp.tile([128, WS], F32, tag="w32", name=f"w32_{i}")
                    for i in range(L)]
            for l in range(L):
                nc.sync.dma_start(wl16[l][:], w16_d.ap()[l])
                nc.sync.dma_start(wl8[l][:], w8_d.ap()[l])
                nc.sync.dma_start(wl32[l][:], w32_d.ap()[l])

            xB8 = sb.tile([128, KC, TR], F8E4)
            xC8 = sb.tile([128, KC, TR], F8E4)

            def tile16(wl, base, idx):
                off = base + idx * 128
                return wl[:, off:off + 128]

            for l in range(L):
                w6, w6_8, w2c = wl16[l], wl8[l], wl32[l]

                if l == 0:
                    # layer 0: x is 0/1 spikes (xl == 0 exactly)
                    nc.vector.tensor_copy(out=xh[:], in_=x_cur[:])
                    nc.scalar.activation(xC8[:], xh[:], AF.Identity,
                                         bias=0.0, scale=1.0)

                # --- gates: xh@Wh (fp16, x2^14) + fp8 DoubleRow corrections
                #     (x-lo)@Wh and xh@(W-lo), K=256 per DR pass ---
                HT = TR // 2
                for half, agx in ((0, aga), (1, agb)):
                    sl = slice(half * HT, (half + 1) * HT)
                    for g in range(3):
                        for hf in range(KC):
                            bank = g * KC + hf
                            passes = [
                                (tile16(w6, GH_OFF, bank * KC + kc),
                                 xh[:, kc, sl]) for kc in range(KC)]
                            if l > 0:
                                passes.append((w6_8[:, 0, bank],
                                               xB8[:, :, sl], DR))
                            passes.append((w6_8[:, 1, bank],
                                           xC8[:, :, sl], DR))
                            _mm16(nc, ps, passes, agx[:, bank, :],
                                  name=f"g{half}{bank}", free=HT,
                                  scale=2.0 ** -14)

                # --- gate LIF scan; per half: spikes, kv, h-recurrence,
                #     rh (as hi/lo via h split: r in {0,1}), Wo matmuls ---
                def ag_src(t):
                    agx = aga if t < 8 else agb
                    tt = t % 8
                    return agx[:, :, tt * R:(tt + 1) * R]

                for t in range(T):
                    nc.vector._custom_dve(
                        lif, out=wg_buf[:, t],
                        in0=(zg[:] if t == 0 else wg_buf[:, t - 1]),
                        in1=ag_src(t), s0=0.5)
                    if t == 7 or t == 15:
                        half = 0 if t == 7 else 1
                        hh = slice(t - 7, t + 1)
                        nc.vector.tensor_scalar(
                            out=s_buf[:, hh], in0=wg_buf[:, hh], scalar1=1.0,
                            scalar2=None, op0=OP.is_ge)
                        nc.vector.tensor_mul(
                            out=kv_buf[:, hh], in0=s_buf[:, hh, 2:4, :],
                            in1=s_buf[:, hh, 4:6, :])
                        for th in range(t - 7, t + 1):
                            nc.vector.scalar_tensor_tensor(
                                out=h_buf[:, th],
                                in0=(zh[:] if th == 0 else h_buf[:, th - 1]),
                                scalar=0.9, in1=kv_buf[:, th],
                                op0=OP.mult, op1=OP.add)
                        # h hi/lo split; rh_hi = r*h_hi, rh_lo = r*h_lo
                        # (exact: r is 0/1)
                        nc.scalar.activation(hh16[:, hh], h_buf[:, hh],
                                             AF.Identity, bias=0.0, scale=1.0)
                        nc.vector.tensor_sub(out=hl16[:, hh],
                                             in0=h_buf[:, hh], in1=hh16[:, hh])
                        nc.vector.tensor_mul(out=rhh[:, hh],
                                             in0=s_buf[:, hh, 0:2, :],
                                             in1=hh16[:, hh])
                        nc.vector.tensor_mul(out=rhl[:, hh],
                                             in0=s_buf[:, hh, 0:2, :],
                                             in1=hl16[:, hh])
                        for hf in range(KC):
                            passes = []
                            for kc in range(KC):
                                wh = tile16(w6, WOH_OFF, hf * KC + kc)
                                wlo = tile16(w6, WOL_OFF, hf * KC + kc)
                                passes += [(wh, rhh[:, hh, kc, :]),
                                           (wh, rhl[:, hh, kc, :]),
                                           (wlo, rhh[:, hh, kc, :])]
                            _mm16(nc, ps, passes,
                                  at_buf[:, hf, half * HT:(half + 1) * HT],
                                  name=f"wo{half}{hf}", free=HT)

                # --- LN1(x + attn) -> x1 and FFN mm1, pipelined per half ---
                for half, a1x in ((0, a1a), (1, a1b)):
                    sl = slice(half * HT, (half + 1) * HT)
                    for kc in range(KC):
                        nc.gpsimd.tensor_add(out=u_buf[:, kc, sl],
                                             in0=x_cur[:, kc, sl],
                                             in1=at_buf[:, kc, sl])
                    _layer_norm(
                        nc, ps, sb, u_buf, sq_buf,
                        lambda kc: w2c[:, LN_OFF + kc:LN_OFF + kc + 1],
                        lambda kc: w2c[:, LN_OFF + 2 + kc:LN_OFF + 2 + kc + 1],
                        lambda kc: x1_buf[:, kc, sl],
                        ones_col, ones_row, eps_col, sl, HT, identity=ln_id)
                    nc.scalar.activation(x1h[:, :, sl], x1_buf[:, :, sl],
                                         AF.Identity, bias=0.0, scale=1.0)
                    nc.vector.tensor_sub(out=x1l[:, :, sl], in0=x1_buf[:, :, sl],
                                         in1=x1h[:, :, sl])
                    nc.scalar.activation(xB8[:, :, sl], x1l[:, :, sl],
                                         AF.Identity, bias=0.0, scale=2.0 ** 11)
                    nc.scalar.activation(xC8[:, :, sl], x1h[:, :, sl],
                                         AF.Identity, bias=0.0, scale=1.0)
                    for mf in range(FC):
                        passes = [(tile16(w6, W1H_OFF, mf * KC + kc),
                                   x1h[:, kc, sl]) for kc in range(KC)]
                        passes.append((w6_8[:, 0, 6 + mf], xB8[:, :, sl], DR))
                        passes.append((w6_8[:, 1, 6 + mf], xC8[:, :, sl], DR))
                        _mm16(nc, ps, passes, a1x[:, mf, :],
                              bias=w2c[:, B1_OFF + mf:B1_OFF + mf + 1],
                              name=f"f{half}{mf}", free=HT, scale=2.0 ** -14)

                # --- LIF1, spikes per half ---
                def a1_src(t):
                    a1x = a1a if t < 8 else a1b
                    tt = t % 8
                    return a1x[:, :, tt * R:(tt + 1) * R]

                for t in range(T):
                    nc.vector._custom_dve(
                        lif, out=w1_buf[:, t],
                        in0=(z1[:] if t == 0 else w1_buf[:, t - 1]),
                        in1=a1_src(t), s0=0.5)
                    if t == 7 or t == 15:
                        hh = slice(t - 7, t + 1)
                        nc.vector.tensor_scalar(
                            out=s1_buf[:, hh], in0=w1_buf[:, hh], scalar1=1.0,
                            scalar2=None, op0=OP.is_ge)

                # --- mm2 (+b2): s1 exact fp16, 2 passes per K chunk, T-split ---
                for half, a2x in ((0, a2a), (1, a2b)):
                    tsl = slice(half * 8, (half + 1) * 8)
                    for mh in range(KC):
                        passes = []
                        for kc8 in range(FC):
                            passes += [
                                (tile16(w6, W2H_OFF, mh * FC + kc8),
                                 s1_buf[:, tsl, kc8, :]),
                                (tile16(w6, W2L_OFF, mh * FC + kc8),
                                 s1_buf[:, tsl, kc8, :]),
                            ]
                        _mm16(nc, ps, passes, a2x[:, mh, :],
                              bias=w2c[:, B2_OFF + mh:B2_OFF + mh + 1],
                              name=f"m2{half}{mh}", free=HT)

                # --- LIF2, spikes per half ---
                def a2_src(t):
                    a2x = a2a if t < 8 else a2b
                    tt = t % 8
                    return a2x[:, :, tt * R:(tt + 1) * R]

                for t in range(T):
                    nc.vector._custom_dve(
                        lif, out=w2_buf[:, t],
                        in0=(zh[:] if t == 0 else w2_buf[:, t - 1]),
                        in1=a2_src(t), s0=0.5)
                    if t == 7 or t == 15:
                        hh = slice(t - 7, t + 1)
                        nc.vector.tensor_scalar(
                            out=s2_buf[:, hh], in0=w2_buf[:, hh], scalar1=1.0,
                            scalar2=None, op0=OP.is_ge)

                # --- LN2(x1 + s2) -> x_cur, per half ---
                for half in (0, 1):
                    sl = slice(half * HT, (half + 1) * HT)
                    tsl = slice(half * 8, (half + 1) * 8)
                    for kc in range(KC):
                        nc.gpsimd.tensor_add(out=u_buf[:, kc, sl],
                                             in0=x1_buf[:, kc, sl],
                                             in1=s2_buf[:, tsl, kc, :])
                    _layer_norm(
                        nc, ps, sb, u_buf, sq_buf,
                        lambda kc: w2c[:, LN_OFF + 4 + kc:LN_OFF + 4 + kc + 1],
                        lambda kc: w2c[:, LN_OFF + 6 + kc:LN_OFF + 6 + kc + 1],
                        lambda kc: x_cur[:, kc, sl],
                        ones_col, ones_row, eps_col, sl, HT, identity=ln_id)
                    if l + 1 < L:
                        nc.scalar.activation(xh[:, :, sl], x_cur[:, :, sl],
                                             AF.Identity, bias=0.0, scale=1.0)
                        nc.vector.tensor_sub(out=xl[:, :, sl],
                                             in0=x_cur[:, :, sl],
                                             in1=xh[:, :, sl])
                        nc.scalar.activation(xB8[:, :, sl], xl[:, :, sl],
                                             AF.Identity, bias=0.0,
                                             scale=2.0 ** 11)
                        nc.scalar.activation(xC8[:, :, sl], xh[:, :, sl],
                                             AF.Identity, bias=0.0, scale=1.0)

            nc.sync.dma_start(h_d.ap()[:], x_cur[:])
    nc.compile()
    return nc


def build_head():
    """Head v2: flipped matmul — h-tiles stationary, Wout streams on the
    free dim. Output layout [tn-rows on partitions, vocab on free].

    Per tn-block (t, n-half): logits*2^14 accumulate in PSUM from 4 passes:
      A (fp16):  hh @ (fp16(W0)*2^14), 2 K-chunks
      B (fp8 DoubleRow, K=256 in 1 pass): e4m3(hl*2^11) @ e4m3(Wh*2^3)
      C (fp8 DoubleRow):                  e4m3(hh)      @ e4m3(Wl*2^14)
    ACT drains with scale 2^-18 (the extra /16 feeds the V = w/16 state
    encoding). The scan is ONE giant fused LIF+count DVE op per (t, n-half)
    on [128 x 4096]: state U = V + count/2 in a single fp32 (see
    _get_lifcnt_op). A final flush step (a = 0) counts the last spike;
    the host reads the count as rint(2U)."""
    lifcnt = _get_lifcnt_op()
    nc = bacc.Bacc("TRN2", target_bir_lowering=False)
    hh_d = nc.dram_tensor("hTh", [128, KC, TN], F16, kind="ExternalInput")
    hb_d = nc.dram_tensor("hB8", [128, KC, TN], F8E4, kind="ExternalInput")
    hc_d = nc.dram_tensor("hC8", [128, KC, TN], F8E4, kind="ExternalInput")
    ws_d = nc.dram_tensor("wS", [128, KC, VSH], F16, kind="ExternalInput")
    wb_d = nc.dram_tensor("wB8", [128, KC, VSH], F8E4, kind="ExternalInput")
    wc_d = nc.dram_tensor("wC8", [128, KC, VSH], F8E4, kind="ExternalInput")
    o_d = nc.dram_tensor("out_nh", [2, 128, VSH], F32, kind="ExternalOutput")

    VB = VSH // 512  # 8 psum-bank columns
    with tile.TileContext(nc) as tc:
        with tc.tile_pool(name="sb", bufs=1) as sb, \
             tc.tile_pool(name="ps", bufs=1, space="PSUM") as ps:

            hh = sb.tile([128, KC, TN], F16)
            hb = sb.tile([128, KC, TN], F8E4)
            hc = sb.tile([128, KC, TN], F8E4)
            ws = sb.tile([128, KC, VSH], F16)
            wb = sb.tile([128, KC, VSH], F8E4)
            wc = sb.tile([128, KC, VSH], F8E4)
            # interleave DMAs so tile-0 operands land first
            QT, QV = TN // 4, VSH // 4
            for q in range(4):
                for kc in range(KC):
                    nc.sync.dma_start(ws[:, kc, q * QV:(q + 1) * QV],
                                      ws_d.ap()[:, kc, q * QV:(q + 1) * QV])
                    nc.sync.dma_start(hh[:, kc, q * QT:(q + 1) * QT],
                                      hh_d.ap()[:, kc, q * QT:(q + 1) * QT])
                    nc.sync.dma_start(wb[:, kc, q * QV:(q + 1) * QV],
                                      wb_d.ap()[:, kc, q * QV:(q + 1) * QV])
                    nc.sync.dma_start(wc[:, kc, q * QV:(q + 1) * QV],
                                      wc_d.ap()[:, kc, q * QV:(q + 1) * QV])
                    nc.sync.dma_start(hb[:, kc, q * QT:(q + 1) * QT],
                                      hb_d.ap()[:, kc, q * QT:(q + 1) * QT])
                    nc.sync.dma_start(hc[:, kc, q * QT:(q + 1) * QT],
                                      hc_d.ap()[:, kc, q * QT:(q + 1) * QT])

            w_st = [sb.tile([128, 32, 128], F32, name=f"wst{nh}")
                    for nh in range(2)]
            zeros = sb.tile([128, 32, 128], F32, name="zeros")
            for nh in range(2):
                nc.vector.memset(w_st[nh][:], 0.0)
            nc.vector.memset(zeros[:], 0.0)

            a_ring = [sb.tile([128, 32, 128], F32, name=f"a{k}")
                      for k in range(4)]

            for t in range(T):
                for nh in range(2):
                    tb = t * 2 + nh
                    slot = a_ring[tb % 4]
                    for vh in range(2):
                        bank = ps.tile([128, 2048], F32, tag="mm",
                                       name=f"mm{tb}_{vh}", bufs=2)
                        hsl = slice(tb * 128, (tb + 1) * 128)
                        # A: fp16 hi passes (pass-outer, bank-inner: one
                        # weight load streams 4 x 512)
                        for kc in range(KC):
                            lhsT = hh[:, kc, hsl]
                            for b in range(4):
                                off = vh * 2048 + b * 512
                                nc.tensor.matmul(
                                    bank[:, b * 512:(b + 1) * 512], lhsT,
                                    ws[:, kc, off:off + 512],
                                    start=(kc == 0), stop=False)
                        # B, C: fp8 DoubleRow, K=256 in one pass each
                        for i, (hsrc, wsrc) in enumerate(((hb, wb), (hc, wc))):
                            lhsT = hsrc[:, :, hsl]
                            for b in range(4):
                                off = vh * 2048 + b * 512
                                nc.tensor.matmul(
                                    bank[:, b * 512:(b + 1) * 512], lhsT,
                                    wsrc[:, :, off:off + 512],
                                    start=False, stop=(i == 1 and b == 3),
                                    perf_mode=DR)
                        nc.scalar.activation(
                            slot[:, vh * 16:(vh + 1) * 16, :], bank[:],
                            AF.Identity, bias=0.0, scale=2.0 ** -18)
                    # fused LIF + count scan step, in-place state
                    nc.vector._custom_dve(
                        lifcnt, out=w_st[nh][:], in0=w_st[nh][:],
                        in1=slot[:], s0=0.25, s1=1.5 * 2.0 ** 22)

            for nh in range(2):
                # flush: one extra step (a=0) counts the final state's spike
                nc.vector._custom_dve(
                    lifcnt, out=w_st[nh][:], in0=w_st[nh][:],
                    in1=zeros[:], s0=0.25, s1=1.5 * 2.0 ** 22)
                nc.sync.dma_start(o_d.ap()[nh], w_st[nh][:])
    nc.compile()
    return nc


_CACHE = {}
TRACE = False
LAST = {}


def _run(nc, in_maps, key):
    import tempfile

    if TRACE:
        td = tempfile.mkdtemp(prefix=f"bkt_{key}_")
        res = run_bass_kernel_spmd(nc, in_maps, core_ids=list(range(NCORE)),
                                   trace=True, tmpdir=td)
        LAST[key] = (res, td)
        return res
    return run_bass_kernel_spmd(nc, in_maps, core_ids=list(range(NCORE)))


def _get_programs(ln_id):
    key = f"blocks{ln_id}"
    if key not in _CACHE:
        _CACHE[key] = build_blocks(ln_id=ln_id)
    if "head" not in _CACHE:
        _CACHE["head"] = build_head()
    return _CACHE[key], _CACHE["head"]


def _pack_weights(Wr, Wk, Wv, Wo, W1, b1, W2, b2, g1, be1, g2, be2):
    import ml_dtypes
    e4t = ml_dtypes.float8_e4m3
    w16 = np.zeros((L, 128, W16), np.float16)
    w8 = np.zeros((L, 128, 2, NB8, 2, 128), e4t)
    w32 = np.zeros((L, 128, WS), np.float32)
    for l in range(L):
        his = []

        def add(mat):  # [K, M] fp32 -> fp16 hi (+ lo for Wo/W2)
            hi, lo = _split16(mat)
            his.append(hi)
            return lo

        # gates + W1: hi fp16 pre-scaled 2^14; B/C corrections fp8 DR tiles
        gh = []
        for bank in range(NB8):
            for kc in range(KC):
                if bank < 6:
                    g, hf = divmod(bank, KC)
                    Wg = (Wr, Wk, Wv)[g]
                    blk = 0.5 * Wg[l][kc * 128:(kc + 1) * 128,
                                      hf * 128:(hf + 1) * 128]
                else:
                    mf = bank - 6
                    blk = 0.5 * W1[l][kc * 128:(kc + 1) * 128,
                                      mf * 128:(mf + 1) * 128]
                hi = blk.astype(np.float16)
                lo = blk - hi.astype(np.float32)
                gh.append((hi.astype(np.float32) * 2.0 ** 14)
                          .astype(np.float16))
                w8[l, :, 0, bank, kc, :] = (hi.astype(np.float32) * 2.0 ** 3
                                            ).astype(e4t)
                w8[l, :, 1, bank, kc, :] = (lo * 2.0 ** 14).astype(e4t)
        ghs = np.concatenate(gh[:12], axis=1)     # gates hi
        w1h = np.concatenate(gh[12:], axis=1)     # W1 hi
        his = []
        los = []
        for hf in range(KC):
            for kc in range(KC):
                los.append(add(Wo[l][kc * 128:(kc + 1) * 128,
                                     hf * 128:(hf + 1) * 128]))
        woh = np.concatenate(his, axis=1)
        wol = np.concatenate([x.astype(np.float16) for x in los], axis=1)
        his, los = [], []
        for mh in range(KC):
            for kc8 in range(FC):
                los.append(add(0.5 * W2[l][kc8 * 128:(kc8 + 1) * 128,
                                           mh * 128:(mh + 1) * 128]))
        w2h = np.concatenate(his, axis=1)
        w2l = np.concatenate([x.astype(np.float16) for x in los], axis=1)
        w16[l] = np.concatenate([ghs, woh, wol, w1h, w2h, w2l], axis=1)
        w32[l] = np.concatenate([
            0.5 * b1[l].reshape(FC, 128).T,
            0.5 * b2[l].reshape(KC, 128).T,
            g1[l].reshape(KC, 128).T, be1[l].reshape(KC, 128).T,
            g2[l].reshape(KC, 128).T, be2[l].reshape(KC, 128).T,
        ], axis=1)
    return (np.ascontiguousarray(w16), np.ascontiguousarray(w8),
            np.ascontiguousarray(w32))


def kernel(input_ids, token_embedding, pos_embedding, noise, unif,
           Wr, Wk, Wv, Wo, W1, b1, W2, b2, ln1_g, ln1_b, ln2_g, ln2_b,
           Wout, bout):
    input_ids = np.asarray(input_ids)
    f32 = lambda a: np.asarray(a, dtype=np.float32)
    token_embedding, pos_embedding, noise, unif = map(
        f32, (token_embedding, pos_embedding, noise, unif))
    Wr, Wk, Wv, Wo, W1, b1, W2, b2 = map(f32, (Wr, Wk, Wv, Wo, W1, b1, W2, b2))
    ln1_g, ln1_b, ln2_g, ln2_b, Wout, bout = map(
        f32, (ln1_g, ln1_b, ln2_g, ln2_b, Wout, bout))

    ln_id = bool((ln1_g == 1).all() and (ln1_b == 0).all()
                 and (ln2_g == 1).all() and (ln2_b == 0).all())
    nc_blocks, nc_head = _get_programs(ln_id)

    spikes = _encode_spikes(input_ids, token_embedding, pos_embedding, noise, unif)
    sp = spikes.reshape(T, NCORE, R, KC, 128)          # (t, core, r, kc, p)
    x0 = np.ascontiguousarray(sp.transpose(1, 4, 3, 0, 2)).reshape(NCORE, 128, KC, TR)
    w16, w8, w32 = _pack_weights(Wr, Wk, Wv, Wo, W1, b1, W2, b2,
                                 ln1_g, ln1_b, ln2_g, ln2_b)
    in1 = [{"x0": x0[c], "w16": w16, "w8": w8, "w32": w32}
           for c in range(NCORE)]
    res1 = _run(nc_blocks, in1, "blocks")
    ho = np.stack([res1.results[c]["h_out"].reshape(128, KC, T, R)
                   for c in range(NCORE)])
    hT = np.ascontiguousarray(ho.transpose(1, 2, 3, 0, 4)).reshape(128, KC, TN)
    import ml_dtypes
    e4 = ml_dtypes.float8_e4m3
    hTh16 = hT.astype(np.float16)
    hTl = hT - hTh16.astype(np.float32)
    hB8 = np.ascontiguousarray((hTl * 2.0 ** 11).astype(e4))
    hC8 = np.ascontiguousarray(hTh16.astype(np.float32).astype(e4))
    hTh16 = np.ascontiguousarray(hTh16)

    assert not np.any(bout), "head kernel assumes bout == 0 (spec fill=zeros)"
    Wp = np.zeros((D, VPAD), np.float32)
    Wp[:, :V] = 0.5 * Wout
    Wph16 = Wp.astype(np.float16)
    Wpl = Wp - Wph16.astype(np.float32)
    WSc = (Wph16.astype(np.float32) * 2.0 ** 14).astype(np.float16)
    WB8 = (Wph16.astype(np.float32) * 2.0 ** 3).astype(e4)
    WC8 = (Wpl * 2.0 ** 14).astype(e4)

    def shard(Wx, c):
        w = Wx[:, c * VSH:(c + 1) * VSH].reshape(KC, 128, VSH)
        return np.ascontiguousarray(w.transpose(1, 0, 2))
    in2 = [{"hTh": hTh16, "hB8": hB8, "hC8": hC8,
            "wS": shard(WSc, c), "wB8": shard(WB8, c), "wC8": shard(WC8, c)}
           for c in range(NCORE)]
    res2 = _run(nc_head, in2, "head")
    # out_nh[nh, p, v] holds U = V + count/2: count = rint(2U).
    # row n = nh*128 + p, vocab col = c*VSH + v
    out_sh = np.stack([res2.results[c]["out_nh"] for c in range(NCORE)])
    out = np.empty((N, VPAD), np.float32)
    for c in range(NCORE):
        for nh in range(2):
            out[nh * 128:(nh + 1) * 128, c * VSH:(c + 1) * VSH] = \
                np.rint(2.0 * out_sh[c, nh].astype(np.float64)).reshape(128, VSH)
    out = out[:, :V].reshape(B, S, V).astype(np.float32)
    return out



# revision 29
# speedup vs baseline: 1.0342x; 1.0342x over previous
"""Trainium2 Bass kernel for nn_AdvancedSpikingChatModel.

Model: spike-encode embeddings -> 6 spiking-transformer blocks (LIF gates +
decaying linear-attention recurrence over T=16) -> LIF output head with
spike-count accumulation over V=32000 vocab.

Strategy (8 NeuronCores, SPMD, two launches):
  Launch 1 (blocks): data-parallel over the 256 folded (b,s) rows, 32/core.
    Features on partitions, (t, row) on the free dim; weights stationary.
  Launch 2 (head): vocab-parallel, 4096 padded cols/core, all 256 rows.

Precision: matmuls run as fp16 hi/lo split passes (x@W = xh@Wh + xl@Wh +
xh@Wl accumulated in fp32 PSUM; dropped xl@Wl term ~2^-22) — fp32-grade
results at the PE's fp16 rate (fp32 matmuls cost ~2.6x on TRN2). The LIF
threshold compare (v >= 1) makes anything coarser (bf16/fp32r) flip spikes.
Spike matrices (0/1) are exact in fp16, so spike-side matmuls use 2 passes.

LIF decay 0.5 folded into weights: w' = 0.5*(min(w,1) - (w>=1)) + a, emitted
as ONE custom DVE op per step; spikes s = (w >= 1) recovered in one batched
GPSIMD pass per scan; spike counts via add-tree (GPSIMD + DVE).
"""

import numpy as np

import concourse.mybir as mybir
import concourse.tile as tile
from concourse import bacc
from concourse.bass_utils import run_bass_kernel_spmd

F32 = mybir.dt.float32
F16 = mybir.dt.float16
F8E4 = mybir.dt.float8e4
OP = mybir.AluOpType
AF = mybir.ActivationFunctionType
DR = mybir.MatmulPerfMode.DoubleRow

B, S, D, T, L, F, V = 2, 128, 256, 16, 6, 1024, 32000
N = B * S
NCORE = 8
R = N // NCORE       # 32 rows/core in launch 1
TR = T * R           # 512
KC = D // 128
FC = F // 128
VPAD = 32768
VSH = VPAD // NCORE  # 4096
VCH = VSH // 128     # 32 chunks
TN = T * N           # 4096
EPS = 1e-5

# fp16 weight slab offsets (fp16 words per partition, per layer).
# Gates/W1 hi tiles are pre-scaled by 2^14 (PSUM scale shared with the
# fp8 DoubleRow correction passes; drained with ACT scale 2^-14).
GH_OFF = 0
WOH_OFF = GH_OFF + 12 * 128
WOL_OFF = WOH_OFF + 4 * 128
W1H_OFF = WOL_OFF + 4 * 128
W2H_OFF = W1H_OFF + 16 * 128
W2L_OFF = W2H_OFF + 16 * 128
W16 = W2L_OFF + 16 * 128
# fp8 slab: [L, 128, 2(B/C), 14 banks, 2 kc, 128] — banks 0-5 gates, 6-13 W1.
# B = e4m3(Wh * 2^3) pairs with x-lo * 2^11; C = e4m3(Wl * 2^14) with x-hi.
NB8 = 14
# fp32 smalls: b1(8) b2(2) ln(8)
B1_OFF = 0
B2_OFF = 8
LN_OFF = 10
WS = 18

_LIF_OP = None
_LIFCNT_OP = None


def _register_op(name, spec):
    from concourse.dve_ops import (
        DveOp, OPS, _SUB_OPCODE_FOR_NAME, CUSTOM_DVE_SPECS)
    from concourse.dve_spec import lower
    from concourse.dve_uop import DveOpSpec

    if name in _SUB_OPCODE_FOR_NAME:
        return next(o for o in OPS if o.name == name)
    op = DveOp(name, spec, subdim=False, uops_sha={})
    row = 1 + len(OPS)
    OPS.append(op)
    _SUB_OPCODE_FOR_NAME[name] = row
    CUSTOM_DVE_SPECS[name] = spec
    for ver in ("v3",):
        s = DveOpSpec(name=name, opcode=row, uops=lower(spec, ver=ver),
                      rd1_en=True)
        op.uops_sha[ver] = s.sha(ver)
    return op


def _get_lif_op():
    """LIF step as a custom DVE op: out = (min(w,1) - (w>=1))*0.5 + a."""
    global _LIF_OP
    if _LIF_OP is None:
        from concourse.dve_spec import Spec, Src0, Src1, C0, One, minn
        body = (minn(Src0, One) - (Src0 >= One)) * C0 + Src1
        _LIF_OP = _register_op("LIF_STEP_ANT", Spec(
            body=body,
            reference=lambda in0, in1, s0, s1, imm2:
                (np.minimum(in0, 1.0) - (in0 >= 1.0)) * s0 + in1,
        ))
    return _LIF_OP


def _get_lifcnt_op():
    """Fused LIF step + spike count, one DVE pass (8 ALU stages).

    State U = V + A/2 in one fp32: V = w/16 in (-0.25, 0.25) is the membrane,
    A the spike count. s0 = 0.25 (latches derive threshold 1/16 = s0^2 and
    decay 0.5 = s0+s0), s1 = 1.5*2^22 (magic: (U+M)-M rounds U to the
    nearest 0.5 multiple = A/2, exact for |V| < 0.25 both signs).
    Spike branch outputs One, halved to +0.5 == one count unit, V reset 0.
    in1 = a/16 (pre-scaled in the PSUM drain)."""
    global _LIFCNT_OP
    if _LIFCNT_OP is None:
        from concourse.dve_spec import (
            Spec, Src0, Src1, C0, C1, One, Latch, select)
        T16 = Latch(C0 * C0)
        Half = Latch(C0 + C0)
        m1 = Src0 + C1
        r = m1 - C1
        V = Src0 - r
        g = V >= T16
        body = select(g, One, V) * Half + (Src1 + r)

        def ref(in0, in1, s0, s1, imm2):
            f32 = np.float32
            in0 = np.asarray(in0, f32)
            in1 = np.asarray(in1, f32)
            r = (in0 + f32(s1)).astype(f32) - f32(s1)
            V = in0 - r
            g = V >= f32(s0) * f32(s0)
            selv = np.where(g, f32(1.0), V)
            return selv * (f32(s0) + f32(s0)) + (in1 + r)

        _LIFCNT_OP = _register_op("LIF_CNT_ANT", Spec(body=body, reference=ref))
    return _LIFCNT_OP


def _sigmoid(x):
    return 1.0 / (1.0 + np.exp(-x))


def _encode_spikes(input_ids, token_embedding, pos_embedding, noise, unif):
    """Host-side rate coding; (0.7*rate + 0.3*temp > 0.5) == rate exactly."""
    emb = token_embedding[input_ids] + pos_embedding[None, :S]
    p = np.clip(_sigmoid(emb) * 0.8 + 0.1 + noise * 0.05, 0.0, 1.0)
    return (unif < p[None]).astype(np.float32)


def _split16(x):
    hi = x.astype(np.float16)
    lo = (x - hi.astype(np.float32)).astype(np.float16)
    return hi, lo


def _mm16(nc, ps, passes, dst_ap, bias=0.0, name="mmb", free=512, scale=1.0):
    """Accumulate matmul passes into one PSUM bank, ACT-copy(+bias) out.
    A pass is (lhsT, rhs) fp16 or (lhsT, rhs, perf_mode) for fp8 DR."""
    bank = ps.tile([128, free], F32, tag="mm", name=name, bufs=4)
    npass = len(passes)
    for i, p in enumerate(passes):
        pm = p[2] if len(p) > 2 else None
        nc.tensor.matmul(bank[:], p[0], p[1],
                         start=(i == 0), stop=(i == npass - 1), perf_mode=pm)
    nc.scalar.activation(dst_ap, bank[:], AF.Identity, bias=bias, scale=scale)


def _w_scan(nc, lif, w_buf, z0, a_fn, nt=T, sliced=False):
    """w_t = (min(w_{t-1},1) - (w_{t-1}>=1))*0.5 + a_t via the custom op.
    in1 must keep >=2 free dims (STT encoding; the TTSS form runs ~10x slower)."""
    for t in range(nt):
        if sliced:
            out = w_buf[:, t:t + 1, :]
            in0 = z0[:] if t == 0 else w_buf[:, t - 1:t, :]
        else:
            out = w_buf[:, t]
            in0 = z0[:] if t == 0 else w_buf[:, t - 1]
        nc.vector._custom_dve(lif, out=out, in0=in0, in1=a_fn(t), s0=0.5)


def _layer_norm(nc, ps, sb, u, sq_buf, gamma_col, beta_col, out_fn,
                ones_col, ones_row, eps_col, csl, W, identity=False):
    """LN over features (partitions x KC chunks) on a column slice csl of
    width W. u: [128, KC, TR] fp32; out_fn(kc) -> dst AP for that slice.
    identity=True skips the gamma/beta affine (gamma==1, beta==0)."""
    for kc in range(KC):
        nc.scalar.activation(sq_buf[:, kc, csl], u[:, kc, csl], AF.Square)
    ps_m = ps.tile([1, W], F32, tag="st", name="ps_m", bufs=2)
    ps_q = ps.tile([1, W], F32, tag="st", name="ps_q", bufs=2)
    for kc in range(KC):
        nc.tensor.matmul(ps_m[:], ones_col[:], u[:, kc, csl],
                         start=(kc == 0), stop=(kc == KC - 1))
    for kc in range(KC):
        nc.tensor.matmul(ps_q[:], ones_col[:], sq_buf[:, kc, csl],
                         start=(kc == 0), stop=(kc == KC - 1))
    m_sb = sb.tile([1, W], F32, name="m_sb", tag="m_sb", bufs=2)
    q_sb = sb.tile([1, W], F32, name="q_sb", tag="q_sb", bufs=2)
    nc.scalar.mul(m_sb[:], ps_m[:], 1.0 / D)
    nc.scalar.mul(q_sb[:], ps_q[:], 1.0 / D)
    ve = sb.tile([1, W], F32, name="ve", tag="ve", bufs=2)
    nc.vector.tensor_mul(out=ve[:], in0=m_sb[:], in1=m_sb[:])
    nc.vector.tensor_sub(out=ve[:], in0=q_sb[:], in1=ve[:])
    # rstd = 1/sqrt(var+eps): ACT sqrt (eps via bias) + fast reciprocal
    r0 = sb.tile([1, W], F32, name="r0", tag="r0", bufs=2)
    nc.scalar.activation(r0[:], ve[:], AF.Sqrt, bias=eps_col[:])
    nc.vector.reciprocal_approx_fast(r0[:], r0[:])
    pb_m = ps.tile([128, W], F32, tag="bc", name="pb_m", bufs=2)
    pb_r = ps.tile([128, W], F32, tag="bc", name="pb_r", bufs=2)
    nc.tensor.matmul(pb_m[:], ones_row[:], m_sb[:], start=True, stop=True)
    nc.tensor.matmul(pb_r[:], ones_row[:], r0[:], start=True, stop=True)
    for kc in range(KC):
        o = out_fn(kc)
        nc.vector.tensor_sub(out=o, in0=u[:, kc, csl], in1=pb_m[:])
        nc.vector.tensor_mul(out=o, in0=o, in1=pb_r[:])
        if not identity:
            nc.vector.tensor_scalar(out=o, in0=o, scalar1=gamma_col(kc),
                                    scalar2=beta_col(kc), op0=OP.mult,
                                    op1=OP.add)


def build_blocks(ln_id=True):
    lif = _get_lif_op()
    nc = bacc.Bacc("TRN2", target_bir_lowering=False)
    x0_d = nc.dram_tensor("x0", [128, KC, TR], F32, kind="ExternalInput")
    w16_d = nc.dram_tensor("w16", [L, 128, W16], F16, kind="ExternalInput")
    w8_d = nc.dram_tensor("w8", [L, 128, 2, NB8, 2, 128], F8E4,
                          kind="ExternalInput")
    w32_d = nc.dram_tensor("w32", [L, 128, WS], F32, kind="ExternalInput")
    h_d = nc.dram_tensor("h_out", [128, KC, TR], F32, kind="ExternalOutput")

    with tile.TileContext(nc) as tc:
        with tc.tile_pool(name="wp", bufs=2) as wp, \
             tc.tile_pool(name="sb", bufs=1) as sb, \
             tc.tile_pool(name="ps", bufs=1, space="PSUM") as ps:

            ones_col = sb.tile([128, 1], F32)
            ones_row = sb.tile([1, 128], F32)
            eps_col = sb.tile([1, 1], F32)
            nc.vector.memset(ones_col[:], 1.0)
            nc.vector.memset(ones_row[:], 1.0)
            nc.vector.memset(eps_col[:], EPS)

            x_cur = sb.tile([128, KC, TR], F32)
            nc.sync.dma_start(x_cur[:], x0_d.ap()[:])

            xh = sb.tile([128, KC, TR], F16)
            xl = sb.tile([128, KC, TR], F16)
            aga = sb.tile([128, 6, TR // 2], F32)
            agb = sb.tile([128, 6, TR // 2], F32)
            wg_buf = sb.tile([128, T, 6, R], F32)
            s_buf = sb.tile([128, T, 6, R], F16)
            kv_buf = sb.tile([128, T, KC, R], F16)
            h_buf = sb.tile([128, T, KC, R], F32)
            hh16 = sb.tile([128, T, KC, R], F16)
            hl16 = sb.tile([128, T, KC, R], F16)
            rhh = sb.tile([128, T, KC, R], F16)
            rhl = sb.tile([128, T, KC, R], F16)
            at_buf = sb.tile([128, KC, TR], F32)
            u_buf = sb.tile([128, KC, TR], F32)
            sq_buf = sb.tile([128, KC, TR], F32)
            x1_buf = sb.tile([128, KC, TR], F32)
            x1h = sb.tile([128, KC, TR], F16)
            x1l = sb.tile([128, KC, TR], F16)
            a1a = sb.tile([128, FC, TR // 2], F32)
            a1b = sb.tile([128, FC, TR // 2], F32)
            w1_buf = sb.tile([128, T, FC, R], F32)
            s1_buf = sb.tile([128, T, FC, R], F16)
            a2a = sb.tile([128, KC, TR // 2], F32)
            a2b = sb.tile([128, KC, TR // 2], F32)
            w2_buf = sb.tile([128, T, KC, R], F32)
            s2_buf = sb.tile([128, T, KC, R], F16)
            zg = sb.tile([128, 6, R], F32)
            zh = sb.tile([128, KC, R], F32)
            z1 = sb.tile([128, FC, R], F32)
            nc.vector.memset(zg[:], 0.0)
            nc.vector.memset(zh[:], 0.0)
            nc.vector.memset(z1[:], 0.0)

            wl16 = [wp.tile([128, W16], F16, tag="w16", name=f"w16_{i}")
                    for i in range(L)]
            wl8 = [wp.tile([128, 2, NB8, 2, 128], F8E4, tag="w8",
                           name=f"w8_{i}") for i in range(L)]
            wl32 = [wp.tile([128, WS], F32, tag="w32", name=f"w32_{i}")
                    for i in range(L)]
            for l in range(L):
                nc.sync.dma_start(wl16[l][:], w16_d.ap()[l])
                nc.sync.dma_start(wl8[l][:], w8_d.ap()[l])
                nc.sync.dma_start(wl32[l][:], w32_d.ap()[l])

            xB8 = sb.tile([128, KC, TR], F8E4)
            xC8 = sb.tile([128, KC, TR], F8E4)

            def tile16(wl, base, idx):
                off = base + idx * 128
                return wl[:, off:off + 128]

            for l in range(L):
                w6, w6_8, w2c = wl16[l], wl8[l], wl32[l]

                if l == 0:
                    # layer 0: x is 0/1 spikes (xl == 0 exactly)
                    nc.vector.tensor_copy(out=xh[:], in_=x_cur[:])
                    nc.scalar.activation(xC8[:], xh[:], AF.Identity,
                                         bias=0.0, scale=1.0)

                # --- gates: xh@Wh (fp16, x2^14) + fp8 DoubleRow corrections
                #     (x-lo)@Wh and xh@(W-lo), K=256 per DR pass ---
                HT = TR // 2
                for half, agx in ((0, aga), (1, agb)):
                    sl = slice(half * HT, (half + 1) * HT)
                    for g in range(3):
                        for hf in range(KC):
                            bank = g * KC + hf
                            passes = [
                                (tile16(w6, GH_OFF, bank * KC + kc),
                                 xh[:, kc, sl]) for kc in range(KC)]
                            if l > 0:
                                passes.append((w6_8[:, 0, bank],
                                               xB8[:, :, sl], DR))
                            passes.append((w6_8[:, 1, bank],
                                           xC8[:, :, sl], DR))
                            _mm16(nc, ps, passes, agx[:, bank, :],
                                  name=f"g{half}{bank}", free=HT,
                                  scale=2.0 ** -14)

                # --- gate LIF scan; per half: spikes, kv, h-recurrence,
                #     rh (as hi/lo via h split: r in {0,1}), Wo matmuls ---
                def ag_src(t):
                    agx = aga if t < 8 else agb
                    tt = t % 8
                    return agx[:, :, tt * R:(tt + 1) * R]

                for t in range(T):
                    nc.vector._custom_dve(
                        lif, out=wg_buf[:, t],
                        in0=(zg[:] if t == 0 else wg_buf[:, t - 1]),
                        in1=ag_src(t), s0=0.5)
                    if t == 7 or t == 15:
                        half = 0 if t == 7 else 1
                        hh = slice(t - 7, t + 1)
                        nc.vector.tensor_scalar(
                            out=s_buf[:, hh], in0=wg_buf[:, hh], scalar1=1.0,
                            scalar2=None, op0=OP.is_ge)
                        nc.vector.tensor_mul(
                            out=kv_buf[:, hh], in0=s_buf[:, hh, 2:4, :],
                            in1=s_buf[:, hh, 4:6, :])
                        for th in range(t - 7, t + 1):
                            nc.vector.scalar_tensor_tensor(
                                out=h_buf[:, th],
                                in0=(zh[:] if th == 0 else h_buf[:, th - 1]),
                                scalar=0.9, in1=kv_buf[:, th],
                                op0=OP.mult, op1=OP.add)
                        # h hi/lo split; rh_hi = r*h_hi, rh_lo = r*h_lo
                        # (exact: r is 0/1)
                        nc.vector.tensor_copy(out=hh16[:, hh], in_=h_buf[:, hh])
                        nc.vector.tensor_sub(out=hl16[:, hh],
                                             in0=h_buf[:, hh], in1=hh16[:, hh])
                        nc.vector.tensor_mul(out=rhh[:, hh],
                                             in0=s_buf[:, hh, 0:2, :],
                                             in1=hh16[:, hh])
                        nc.vector.tensor_mul(out=rhl[:, hh],
                                             in0=s_buf[:, hh, 0:2, :],
                                             in1=hl16[:, hh])
                        for hf in range(KC):
                            passes = []
                            for kc in range(KC):
                                wh = tile16(w6, WOH_OFF, hf * KC + kc)
                                wlo = tile16(w6, WOL_OFF, hf * KC + kc)
                                passes += [(wh, rhh[:, hh, kc, :]),
                                           (wh, rhl[:, hh, kc, :]),
                                           (wlo, rhh[:, hh, kc, :])]
                            _mm16(nc, ps, passes,
                                  at_buf[:, hf, half * HT:(half + 1) * HT],
                                  name=f"wo{half}{hf}", free=HT)

                # --- LN1(x + attn) -> x1 and FFN mm1, pipelined per half ---
                for half, a1x in ((0, a1a), (1, a1b)):
                    sl = slice(half * HT, (half + 1) * HT)
                    for kc in range(KC):
                        nc.vector.tensor_add(out=u_buf[:, kc, sl],
                                             in0=x_cur[:, kc, sl],
                                             in1=at_buf[:, kc, sl])
                    _layer_norm(
                        nc, ps, sb, u_buf, sq_buf,
                        lambda kc: w2c[:, LN_OFF + kc:LN_OFF + kc + 1],
                        lambda kc: w2c[:, LN_OFF + 2 + kc:LN_OFF + 2 + kc + 1],
                        lambda kc: x1_buf[:, kc, sl],
                        ones_col, ones_row, eps_col, sl, HT, identity=ln_id)
                    nc.vector.tensor_copy(out=x1h[:, :, sl], in_=x1_buf[:, :, sl])
                    nc.vector.tensor_sub(out=x1l[:, :, sl], in0=x1_buf[:, :, sl],
                                         in1=x1h[:, :, sl])
                    nc.scalar.activation(xB8[:, :, sl], x1l[:, :, sl],
                                         AF.Identity, bias=0.0, scale=2.0 ** 11)
                    nc.scalar.activation(xC8[:, :, sl], x1h[:, :, sl],
                                         AF.Identity, bias=0.0, scale=1.0)
                    for mf in range(FC):
                        passes = [(tile16(w6, W1H_OFF, mf * KC + kc),
                                   x1h[:, kc, sl]) for kc in range(KC)]
                        passes.append((w6_8[:, 0, 6 + mf], xB8[:, :, sl], DR))
                        passes.append((w6_8[:, 1, 6 + mf], xC8[:, :, sl], DR))
                        _mm16(nc, ps, passes, a1x[:, mf, :],
                              bias=w2c[:, B1_OFF + mf:B1_OFF + mf + 1],
                              name=f"f{half}{mf}", free=HT, scale=2.0 ** -14)

                # --- LIF1, spikes per half ---
                def a1_src(t):
                    a1x = a1a if t < 8 else a1b
                    tt = t % 8
                    return a1x[:, :, tt * R:(tt + 1) * R]

                for t in range(T):
                    nc.vector._custom_dve(
                        lif, out=w1_buf[:, t],
                        in0=(z1[:] if t == 0 else w1_buf[:, t - 1]),
                        in1=a1_src(t), s0=0.5)
                    if t == 7 or t == 15:
                        hh = slice(t - 7, t + 1)
                        nc.vector.tensor_scalar(
                            out=s1_buf[:, hh], in0=w1_buf[:, hh], scalar1=1.0,
                            scalar2=None, op0=OP.is_ge)

                # --- mm2 (+b2): s1 exact fp16, 2 passes per K chunk, T-split ---
                for half, a2x in ((0, a2a), (1, a2b)):
                    tsl = slice(half * 8, (half + 1) * 8)
                    for mh in range(KC):
                        passes = []
                        for kc8 in range(FC):
                            passes += [
                                (tile16(w6, W2H_OFF, mh * FC + kc8),
                                 s1_buf[:, tsl, kc8, :]),
                                (tile16(w6, W2L_OFF, mh * FC + kc8),
                                 s1_buf[:, tsl, kc8, :]),
                            ]
                        _mm16(nc, ps, passes, a2x[:, mh, :],
                              bias=w2c[:, B2_OFF + mh:B2_OFF + mh + 1],
                              name=f"m2{half}{mh}", free=HT)

                # --- LIF2, spikes per half ---
                def a2_src(t):
                    a2x = a2a if t < 8 else a2b
                    tt = t % 8
                    return a2x[:, :, tt * R:(tt + 1) * R]

                for t in range(T):
                    nc.vector._custom_dve(
                        lif, out=w2_buf[:, t],
                        in0=(zh[:] if t == 0 else w2_buf[:, t - 1]),
                        in1=a2_src(t), s0=0.5)
                    if t == 7 or t == 15:
                        hh = slice(t - 7, t + 1)
                        nc.vector.tensor_scalar(
                            out=s2_buf[:, hh], in0=w2_buf[:, hh], scalar1=1.0,
                            scalar2=None, op0=OP.is_ge)

                # --- LN2(x1 + s2) -> x_cur, per half ---
                for half in (0, 1):
                    sl = slice(half * HT, (half + 1) * HT)
                    tsl = slice(half * 8, (half + 1) * 8)
                    for kc in range(KC):
                        nc.vector.tensor_add(out=u_buf[:, kc, sl],
                                             in0=x1_buf[:, kc, sl],
                                             in1=s2_buf[:, tsl, kc, :])
                    _layer_norm(
                        nc, ps, sb, u_buf, sq_buf,
                        lambda kc: w2c[:, LN_OFF + 4 + kc:LN_OFF + 4 + kc + 1],
                        lambda kc: w2c[:, LN_OFF + 6 + kc:LN_OFF + 6 + kc + 1],
                        lambda kc: x_cur[:, kc, sl],
                        ones_col, ones_row, eps_col, sl, HT, identity=ln_id)
                    if l + 1 < L:
                        nc.vector.tensor_copy(out=xh[:, :, sl],
                                              in_=x_cur[:, :, sl])
                        nc.vector.tensor_sub(out=xl[:, :, sl],
                                             in0=x_cur[:, :, sl],
                                             in1=xh[:, :, sl])
                        nc.scalar.activation(xB8[:, :, sl], xl[:, :, sl],
                                             AF.Identity, bias=0.0,
                                             scale=2.0 ** 11)
                        nc.scalar.activation(xC8[:, :, sl], xh[:, :, sl],
                                             AF.Identity, bias=0.0, scale=1.0)

            nc.sync.dma_start(h_d.ap()[:], x_cur[:])
    nc.compile()
    return nc


def build_head():
    """Head v2: flipped matmul — h-tiles stationary, Wout streams on the
    free dim. Output layout [tn-rows on partitions, vocab on free].

    Per tn-block (t, n-half): logits*2^14 accumulate in PSUM from 4 passes:
      A (fp16):  hh @ (fp16(W0)*2^14), 2 K-chunks
      B (fp8 DoubleRow, K=256 in 1 pass): e4m3(hl*2^11) @ e4m3(Wh*2^3)
      C (fp8 DoubleRow):                  e4m3(hh)      @ e4m3(Wl*2^14)
    ACT drains with scale 2^-18 (the extra /16 feeds the V = w/16 state
    encoding). The scan is ONE giant fused LIF+count DVE op per (t, n-half)
    on [128 x 4096]: state U = V + count/2 in a single fp32 (see
    _get_lifcnt_op). A final flush step (a = 0) counts the last spike;
    the host reads the count as rint(2U)."""
    lifcnt = _get_lifcnt_op()
    nc = bacc.Bacc("TRN2", target_bir_lowering=False)
    hh_d = nc.dram_tensor("hTh", [128, KC, TN], F16, kind="ExternalInput")
    hb_d = nc.dram_tensor("hB8", [128, KC, TN], F8E4, kind="ExternalInput")
    hc_d = nc.dram_tensor("hC8", [128, KC, TN], F8E4, kind="ExternalInput")
    ws_d = nc.dram_tensor("wS", [128, KC, VSH], F16, kind="ExternalInput")
    wb_d = nc.dram_tensor("wB8", [128, KC, VSH], F8E4, kind="ExternalInput")
    wc_d = nc.dram_tensor("wC8", [128, KC, VSH], F8E4, kind="ExternalInput")
    o_d = nc.dram_tensor("out_nh", [2, 128, VSH], F32, kind="ExternalOutput")

    VB = VSH // 512  # 8 psum-bank columns
    with tile.TileContext(nc) as tc:
        with tc.tile_pool(name="sb", bufs=1) as sb, \
             tc.tile_pool(name="ps", bufs=1, space="PSUM") as ps:

            hh = sb.tile([128, KC, TN], F16)
            hb = sb.tile([128, KC, TN], F8E4)
            hc = sb.tile([128, KC, TN], F8E4)
            ws = sb.tile([128, KC, VSH], F16)
            wb = sb.tile([128, KC, VSH], F8E4)
            wc = sb.tile([128, KC, VSH], F8E4)
            # interleave DMAs so tile-0 operands land first
            QT, QV = TN // 4, VSH // 4
            for q in range(4):
                for kc in range(KC):
                    nc.sync.dma_start(ws[:, kc, q * QV:(q + 1) * QV],
                                      ws_d.ap()[:, kc, q * QV:(q + 1) * QV])
                    nc.sync.dma_start(hh[:, kc, q * QT:(q + 1) * QT],
                                      hh_d.ap()[:, kc, q * QT:(q + 1) * QT])
                    nc.sync.dma_start(wb[:, kc, q * QV:(q + 1) * QV],
                                      wb_d.ap()[:, kc, q * QV:(q + 1) * QV])
                    nc.sync.dma_start(wc[:, kc, q * QV:(q + 1) * QV],
                                      wc_d.ap()[:, kc, q * QV:(q + 1) * QV])
                    nc.sync.dma_start(hb[:, kc, q * QT:(q + 1) * QT],
                                      hb_d.ap()[:, kc, q * QT:(q + 1) * QT])
                    nc.sync.dma_start(hc[:, kc, q * QT:(q + 1) * QT],
                                      hc_d.ap()[:, kc, q * QT:(q + 1) * QT])

            w_st = [sb.tile([128, 32, 128], F32, name=f"wst{nh}")
                    for nh in range(2)]
            zeros = sb.tile([128, 32, 128], F32, name="zeros")
            for nh in range(2):
                nc.vector.memset(w_st[nh][:], 0.0)
            nc.vector.memset(zeros[:], 0.0)

            a_ring = [sb.tile([128, 32, 128], F32, name=f"a{k}")
                      for k in range(4)]

            for t in range(T):
                for nh in range(2):
                    tb = t * 2 + nh
                    slot = a_ring[tb % 4]
                    for vh in range(2):
                        bank = ps.tile([128, 2048], F32, tag="mm",
                                       name=f"mm{tb}_{vh}", bufs=2)
                        hsl = slice(tb * 128, (tb + 1) * 128)
                        # A: fp16 hi passes (pass-outer, bank-inner: one
                        # weight load streams 4 x 512)
                        for kc in range(KC):
                            lhsT = hh[:, kc, hsl]
                            for b in range(4):
                                off = vh * 2048 + b * 512
                                nc.tensor.matmul(
                                    bank[:, b * 512:(b + 1) * 512], lhsT,
                                    ws[:, kc, off:off + 512],
                                    start=(kc == 0), stop=False)
                        # B, C: fp8 DoubleRow, K=256 in one pass each
                        for i, (hsrc, wsrc) in enumerate(((hb, wb), (hc, wc))):
                            lhsT = hsrc[:, :, hsl]
                            for b in range(4):
                                off = vh * 2048 + b * 512
                                nc.tensor.matmul(
                                    bank[:, b * 512:(b + 1) * 512], lhsT,
                                    wsrc[:, :, off:off + 512],
                                    start=False, stop=(i == 1 and b == 3),
                                    perf_mode=DR)
                        nc.scalar.activation(
                            slot[:, vh * 16:(vh + 1) * 16, :], bank[:],
                            AF.Identity, bias=0.0, scale=2.0 ** -18)
                    # fused LIF + count scan step, in-place state
                    nc.vector._custom_dve(
                        lifcnt, out=w_st[nh][:], in0=w_st[nh][:],
                        in1=slot[:], s0=0.25, s1=1.5 * 2.0 ** 22)

            for nh in range(2):
                # flush: one extra step (a=0) counts the final state's spike
                nc.vector._custom_dve(
                    lifcnt, out=w_st[nh][:], in0=w_st[nh][:],
                    in1=zeros[:], s0=0.25, s1=1.5 * 2.0 ** 22)
                nc.sync.dma_start(o_d.ap()[nh], w_st[nh][:])
    nc.compile()
    return nc


_CACHE = {}
TRACE = False
LAST = {}


def _run(nc, in_maps, key):
    import tempfile

    if TRACE:
        td = tempfile.mkdtemp(prefix=f"bkt_{key}_")
        res = run_bass_kernel_spmd(nc, in_maps, core_ids=list(range(NCORE)),
                                   trace=True, tmpdir=td)
        LAST[key] = (res, td)
        return res
    return run_bass_kernel_spmd(nc, in_maps, core_ids=list(range(NCORE)))


def _get_programs(ln_id):
    key = f"blocks{ln_id}"
    if key not in _CACHE:
        _CACHE[key] = build_blocks(ln_id=ln_id)
    if "head" not in _CACHE:
        _CACHE["head"] = build_head()
    return _CACHE[key], _CACHE["head"]


def _pack_weights(Wr, Wk, Wv, Wo, W1, b1, W2, b2, g1, be1, g2, be2):
    import ml_dtypes
    e4t = ml_dtypes.float8_e4m3
    w16 = np.zeros((L, 128, W16), np.float16)
    w8 = np.zeros((L, 128, 2, NB8, 2, 128), e4t)
    w32 = np.zeros((L, 128, WS), np.float32)
    for l in range(L):
        his = []

        def add(mat):  # [K, M] fp32 -> fp16 hi (+ lo for Wo/W2)
            hi, lo = _split16(mat)
            his.append(hi)
            return lo

        # gates + W1: hi fp16 pre-scaled 2^14; B/C corrections fp8 DR tiles
        gh = []
        for bank in range(NB8):
            for kc in range(KC):
                if bank < 6:
                    g, hf = divmod(bank, KC)
                    Wg = (Wr, Wk, Wv)[g]
                    blk = 0.5 * Wg[l][kc * 128:(kc + 1) * 128,
                                      hf * 128:(hf + 1) * 128]
                else:
                    mf = bank - 6
                    blk = 0.5 * W1[l][kc * 128:(kc + 1) * 128,
                                      mf * 128:(mf + 1) * 128]
                hi = blk.astype(np.float16)
                lo = blk - hi.astype(np.float32)
                gh.append((hi.astype(np.float32) * 2.0 ** 14)
                          .astype(np.float16))
                w8[l, :, 0, bank, kc, :] = (hi.astype(np.float32) * 2.0 ** 3
                                            ).astype(e4t)
                w8[l, :, 1, bank, kc, :] = (lo * 2.0 ** 14).astype(e4t)
        ghs = np.concatenate(gh[:12], axis=1)     # gates hi
        w1h = np.concatenate(gh[12:], axis=1)     # W1 hi
        his = []
        los = []
        for hf in range(KC):
            for kc in range(KC):
                los.append(add(Wo[l][kc * 128:(kc + 1) * 128,
                                     hf * 128:(hf + 1) * 128]))
        woh = np.concatenate(his, axis=1)
        wol = np.concatenate([x.astype(np.float16) for x in los], axis=1)
        his, los = [], []
        for mh in range(KC):
            for kc8 in range(FC):
                los.append(add(0.5 * W2[l][kc8 * 128:(kc8 + 1) * 128,
                                           mh * 128:(mh + 1) * 128]))
        w2h = np.concatenate(his, axis=1)
        w2l = np.concatenate([x.astype(np.float16) for x in los], axis=1)
        w16[l] = np.concatenate([ghs, woh, wol, w1h, w2h, w2l], axis=1)
        w32[l] = np.concatenate([
            0.5 * b1[l].reshape(FC, 128).T,
            0.5 * b2[l].reshape(KC, 128).T,
            g1[l].reshape(KC, 128).T, be1[l].reshape(KC, 128).T,
            g2[l].reshape(KC, 128).T, be2[l].reshape(KC, 128).T,
        ], axis=1)
    return (np.ascontiguousarray(w16), np.ascontiguousarray(w8),
            np.ascontiguousarray(w32))


def kernel(input_ids, token_embedding, pos_embedding, noise, unif,
           Wr, Wk, Wv, Wo, W1, b1, W2, b2, ln1_g, ln1_b, ln2_g, ln2_b,
           Wout, bout):
    input_ids = np.asarray(input_ids)
    f32 = lambda a: np.asarray(a, dtype=np.float32)
    token_embedding, pos_embedding, noise, unif = map(
        f32, (token_embedding, pos_embedding, noise, unif))
    Wr, Wk, Wv, Wo, W1, b1, W2, b2 = map(f32, (Wr, Wk, Wv, Wo, W1, b1, W2, b2))
    ln1_g, ln1_b, ln2_g, ln2_b, Wout, bout = map(
        f32, (ln1_g, ln1_b, ln2_g, ln2_b, Wout, bout))

    ln_id = bool((ln1_g == 1).all() and (ln1_b == 0).all()
                 and (ln2_g == 1).all() and (ln2_b == 0).all())
    nc_blocks, nc_head = _get_programs(ln_id)

    spikes = _encode_spikes(input_ids, token_embedding, pos_embedding, noise, unif)
    sp = spikes.reshape(T, NCORE, R, KC, 128)          # (t, core, r, kc, p)
    x0 = np.ascontiguousarray(sp.transpose(1, 4, 3, 0, 2)).reshape(NCORE, 128, KC, TR)
    w16, w8, w32 = _pack_weights(Wr, Wk, Wv, Wo, W1, b1, W2, b2,
                                 ln1_g, ln1_b, ln2_g, ln2_b)
    in1 = [{"x0": x0[c], "w16": w16, "w8": w8, "w32": w32}
           for c in range(NCORE)]
    res1 = _run(nc_blocks, in1, "blocks")
    ho = np.stack([res1.results[c]["h_out"].reshape(128, KC, T, R)
                   for c in range(NCORE)])
    hT = np.ascontiguousarray(ho.transpose(1, 2, 3, 0, 4)).reshape(128, KC, TN)
    import ml_dtypes
    e4 = ml_dtypes.float8_e4m3
    hTh16 = hT.astype(np.float16)
    hTl = hT - hTh16.astype(np.float32)
    hB8 = np.ascontiguousarray((hTl * 2.0 ** 11).astype(e4))
    hC8 = np.ascontiguousarray(hTh16.astype(np.float32).astype(e4))
    hTh16 = np.ascontiguousarray(hTh16)

    assert not np.any(bout), "head kernel assumes bout == 0 (spec fill=zeros)"
    Wp = np.zeros((D, VPAD), np.float32)
    Wp[:, :V] = 0.5 * Wout
    Wph16 = Wp.astype(np.float16)
    Wpl = Wp - Wph16.astype(np.float32)
    WSc = (Wph16.astype(np.float32) * 2.0 ** 14).astype(np.float16)
    WB8 = (Wph16.astype(np.float32) * 2.0 ** 3).astype(e4)
    WC8 = (Wpl * 2.0 ** 14).astype(e4)

    def shard(Wx, c):
        w = Wx[:, c * VSH:(c + 1) * VSH].reshape(KC, 128, VSH)
        return np.ascontiguousarray(w.transpose(1, 0, 2))
    in2 = [{"hTh": hTh16, "hB8": hB8, "hC8": hC8,
            "wS": shard(WSc, c), "wB8": shard(WB8, c), "wC8": shard(WC8, c)}
           for c in range(NCORE)]
    res2 = _run(nc_head, in2, "head")
    # out_nh[nh, p, v] holds U = V + count/2: count = rint(2U).
    # row n = nh*128 + p, vocab col = c*VSH + v
    out_sh = np.stack([res2.results[c]["out_nh"] for c in range(NCORE)])
    out = np.empty((N, VPAD), np.float32)
    for c in range(NCORE):
        for nh in range(2):
            out[nh * 128:(nh + 1) * 128, c * VSH:(c + 1) * VSH] = \
                np.rint(2.0 * out_sh[c, nh].astype(np.float64)).reshape(128, VSH)
    out = out[:, :V].reshape(B, S, V).astype(np.float32)
    return out



# revision 30
# speedup vs baseline: 1.0481x; 1.0135x over previous
"""Trainium2 Bass kernel for nn_AdvancedSpikingChatModel.

Model: spike-encode embeddings -> 6 spiking-transformer blocks (LIF gates +
decaying linear-attention recurrence over T=16) -> LIF output head with
spike-count accumulation over V=32000 vocab.

Strategy (8 NeuronCores, SPMD, two launches):
  Launch 1 (blocks): data-parallel over the 256 folded (b,s) rows, 32/core.
    Features on partitions, (t, row) on the free dim; weights stationary.
  Launch 2 (head): vocab-parallel, 4096 padded cols/core, all 256 rows.

Precision: matmuls run as fp16 hi/lo split passes (x@W = xh@Wh + xl@Wh +
xh@Wl accumulated in fp32 PSUM; dropped xl@Wl term ~2^-22) — fp32-grade
results at the PE's fp16 rate (fp32 matmuls cost ~2.6x on TRN2). The LIF
threshold compare (v >= 1) makes anything coarser (bf16/fp32r) flip spikes.
Spike matrices (0/1) are exact in fp16, so spike-side matmuls use 2 passes.

LIF decay 0.5 folded into weights: w' = 0.5*(min(w,1) - (w>=1)) + a, emitted
as ONE custom DVE op per step; spikes s = (w >= 1) recovered in one batched
GPSIMD pass per scan; spike counts via add-tree (GPSIMD + DVE).
"""

import numpy as np

import concourse.mybir as mybir
import concourse.tile as tile
from concourse import bacc
from concourse.bass_utils import run_bass_kernel_spmd

F32 = mybir.dt.float32
F16 = mybir.dt.float16
F8E4 = mybir.dt.float8e4
OP = mybir.AluOpType
AF = mybir.ActivationFunctionType
DR = mybir.MatmulPerfMode.DoubleRow

B, S, D, T, L, F, V = 2, 128, 256, 16, 6, 1024, 32000
N = B * S
NCORE = 8
R = N // NCORE       # 32 rows/core in launch 1
TR = T * R           # 512
KC = D // 128
FC = F // 128
VPAD = 32768
VSH = VPAD // NCORE  # 4096
VCH = VSH // 128     # 32 chunks
TN = T * N           # 4096
EPS = 1e-5

# fp16 weight slab offsets (fp16 words per partition, per layer).
# Gates/W1 hi tiles are pre-scaled by 2^14 (PSUM scale shared with the
# fp8 DoubleRow correction passes; drained with ACT scale 2^-14).
GH_OFF = 0
WOH_OFF = GH_OFF + 12 * 128
WOL_OFF = WOH_OFF + 4 * 128
W1H_OFF = WOL_OFF + 4 * 128
W2H_OFF = W1H_OFF + 16 * 128
W2L_OFF = W2H_OFF + 16 * 128
W16 = W2L_OFF + 16 * 128
# fp8 slab: [L, 128, 2(B/C), 14 banks, 2 kc, 128] — banks 0-5 gates, 6-13 W1.
# B = e4m3(Wh * 2^3) pairs with x-lo * 2^11; C = e4m3(Wl * 2^14) with x-hi.
NB8 = 14
# fp32 smalls: b1(8) b2(2) ln(8)
B1_OFF = 0
B2_OFF = 8
LN_OFF = 10
WS = 18

_LIF_OP = None
_LIFCNT_OP = None


def _register_op(name, spec):
    from concourse.dve_ops import (
        DveOp, OPS, _SUB_OPCODE_FOR_NAME, CUSTOM_DVE_SPECS)
    from concourse.dve_spec import lower
    from concourse.dve_uop import DveOpSpec

    if name in _SUB_OPCODE_FOR_NAME:
        return next(o for o in OPS if o.name == name)
    op = DveOp(name, spec, subdim=False, uops_sha={})
    row = 1 + len(OPS)
    OPS.append(op)
    _SUB_OPCODE_FOR_NAME[name] = row
    CUSTOM_DVE_SPECS[name] = spec
    for ver in ("v3",):
        s = DveOpSpec(name=name, opcode=row, uops=lower(spec, ver=ver),
                      rd1_en=True)
        op.uops_sha[ver] = s.sha(ver)
    return op


def _get_lif_op():
    """LIF step as a custom DVE op: out = (min(w,1) - (w>=1))*0.5 + a."""
    global _LIF_OP
    if _LIF_OP is None:
        from concourse.dve_spec import Spec, Src0, Src1, C0, One, minn
        body = (minn(Src0, One) - (Src0 >= One)) * C0 + Src1
        _LIF_OP = _register_op("LIF_STEP_ANT", Spec(
            body=body,
            reference=lambda in0, in1, s0, s1, imm2:
                (np.minimum(in0, 1.0) - (in0 >= 1.0)) * s0 + in1,
        ))
    return _LIF_OP


def _get_lifcnt_op():
    """Fused LIF step + spike count, one DVE pass (8 ALU stages).

    State U = V + A/2 in one fp32: V = w/16 in (-0.25, 0.25) is the membrane,
    A the spike count. s0 = 0.25 (latches derive threshold 1/16 = s0^2 and
    decay 0.5 = s0+s0), s1 = 1.5*2^22 (magic: (U+M)-M rounds U to the
    nearest 0.5 multiple = A/2, exact for |V| < 0.25 both signs).
    Spike branch outputs One, halved to +0.5 == one count unit, V reset 0.
    in1 = a/16 (pre-scaled in the PSUM drain)."""
    global _LIFCNT_OP
    if _LIFCNT_OP is None:
        from concourse.dve_spec import (
            Spec, Src0, Src1, C0, C1, One, Latch, select)
        T16 = Latch(C0 * C0)
        Half = Latch(C0 + C0)
        m1 = Src0 + C1
        r = m1 - C1
        V = Src0 - r
        g = V >= T16
        body = select(g, One, V) * Half + (Src1 + r)

        def ref(in0, in1, s0, s1, imm2):
            f32 = np.float32
            in0 = np.asarray(in0, f32)
            in1 = np.asarray(in1, f32)
            r = (in0 + f32(s1)).astype(f32) - f32(s1)
            V = in0 - r
            g = V >= f32(s0) * f32(s0)
            selv = np.where(g, f32(1.0), V)
            return selv * (f32(s0) + f32(s0)) + (in1 + r)

        _LIFCNT_OP = _register_op("LIF_CNT_ANT", Spec(body=body, reference=ref))
    return _LIFCNT_OP


def _sigmoid(x):
    return 1.0 / (1.0 + np.exp(-x))


def _encode_spikes(input_ids, token_embedding, pos_embedding, noise, unif):
    """Host-side rate coding; (0.7*rate + 0.3*temp > 0.5) == rate exactly."""
    emb = token_embedding[input_ids] + pos_embedding[None, :S]
    p = np.clip(_sigmoid(emb) * 0.8 + 0.1 + noise * 0.05, 0.0, 1.0)
    return (unif < p[None]).astype(np.float32)


def _split16(x):
    hi = x.astype(np.float16)
    lo = (x - hi.astype(np.float32)).astype(np.float16)
    return hi, lo


def _mm16(nc, ps, passes, dst_ap, bias=0.0, name="mmb", free=512, scale=1.0):
    """Accumulate matmul passes into one PSUM bank, ACT-copy(+bias) out.
    A pass is (lhsT, rhs) fp16 or (lhsT, rhs, perf_mode) for fp8 DR."""
    bank = ps.tile([128, free], F32, tag="mm", name=name, bufs=4)
    npass = len(passes)
    for i, p in enumerate(passes):
        pm = p[2] if len(p) > 2 else None
        nc.tensor.matmul(bank[:], p[0], p[1],
                         start=(i == 0), stop=(i == npass - 1), perf_mode=pm)
    nc.scalar.activation(dst_ap, bank[:], AF.Identity, bias=bias, scale=scale)


def _w_scan(nc, lif, w_buf, z0, a_fn, nt=T, sliced=False):
    """w_t = (min(w_{t-1},1) - (w_{t-1}>=1))*0.5 + a_t via the custom op.
    in1 must keep >=2 free dims (STT encoding; the TTSS form runs ~10x slower)."""
    for t in range(nt):
        if sliced:
            out = w_buf[:, t:t + 1, :]
            in0 = z0[:] if t == 0 else w_buf[:, t - 1:t, :]
        else:
            out = w_buf[:, t]
            in0 = z0[:] if t == 0 else w_buf[:, t - 1]
        nc.vector._custom_dve(lif, out=out, in0=in0, in1=a_fn(t), s0=0.5)


def _layer_norm(nc, ps, sb, u, sq_buf, gamma_col, beta_col, out_fn,
                ones_col, ones_row, eps_col, csl, W, identity=False):
    """LN over features (partitions x KC chunks) on a column slice csl of
    width W. u: [128, KC, TR] fp32; out_fn(kc) -> dst AP for that slice.
    identity=True skips the gamma/beta affine (gamma==1, beta==0)."""
    for kc in range(KC):
        nc.scalar.activation(sq_buf[:, kc, csl], u[:, kc, csl], AF.Square)
    ps_m = ps.tile([1, W], F32, tag="st", name="ps_m", bufs=2)
    ps_q = ps.tile([1, W], F32, tag="st", name="ps_q", bufs=2)
    for kc in range(KC):
        nc.tensor.matmul(ps_m[:], ones_col[:], u[:, kc, csl],
                         start=(kc == 0), stop=(kc == KC - 1))
    for kc in range(KC):
        nc.tensor.matmul(ps_q[:], ones_col[:], sq_buf[:, kc, csl],
                         start=(kc == 0), stop=(kc == KC - 1))
    m_sb = sb.tile([1, W], F32, name="m_sb", tag="m_sb", bufs=2)
    q_sb = sb.tile([1, W], F32, name="q_sb", tag="q_sb", bufs=2)
    nc.scalar.mul(m_sb[:], ps_m[:], 1.0 / D)
    nc.scalar.mul(q_sb[:], ps_q[:], 1.0 / D)
    ve = sb.tile([1, W], F32, name="ve", tag="ve", bufs=2)
    nc.vector.tensor_mul(out=ve[:], in0=m_sb[:], in1=m_sb[:])
    nc.vector.tensor_sub(out=ve[:], in0=q_sb[:], in1=ve[:])
    # rstd = 1/sqrt(var+eps): ACT sqrt (eps via bias) + fast reciprocal
    r0 = sb.tile([1, W], F32, name="r0", tag="r0", bufs=2)
    nc.scalar.activation(r0[:], ve[:], AF.Sqrt, bias=eps_col[:])
    nc.vector.reciprocal_approx_fast(r0[:], r0[:])
    pb_m = ps.tile([128, W], F32, tag="bc", name="pb_m", bufs=2)
    pb_r = ps.tile([128, W], F32, tag="bc", name="pb_r", bufs=2)
    nc.tensor.matmul(pb_m[:], ones_row[:], m_sb[:], start=True, stop=True)
    nc.tensor.matmul(pb_r[:], ones_row[:], r0[:], start=True, stop=True)
    for kc in range(KC):
        o = out_fn(kc)
        nc.vector.tensor_sub(out=o, in0=u[:, kc, csl], in1=pb_m[:])
        nc.vector.tensor_mul(out=o, in0=o, in1=pb_r[:])
        if not identity:
            nc.vector.tensor_scalar(out=o, in0=o, scalar1=gamma_col(kc),
                                    scalar2=beta_col(kc), op0=OP.mult,
                                    op1=OP.add)


def build_blocks(ln_id=True):
    lif = _get_lif_op()
    nc = bacc.Bacc("TRN2", target_bir_lowering=False)
    x0_d = nc.dram_tensor("x0", [128, KC, TR], F32, kind="ExternalInput")
    w16_d = nc.dram_tensor("w16", [L, 128, W16], F16, kind="ExternalInput")
    w8_d = nc.dram_tensor("w8", [L, 128, 2, NB8, 2, 128], F8E4,
                          kind="ExternalInput")
    w32_d = nc.dram_tensor("w32", [L, 128, WS], F32, kind="ExternalInput")
    h_d = nc.dram_tensor("h_out", [128, KC, TR], F32, kind="ExternalOutput")

    with tile.TileContext(nc) as tc:
        with tc.tile_pool(name="wp", bufs=2) as wp, \
             tc.tile_pool(name="sb", bufs=1) as sb, \
             tc.tile_pool(name="ps", bufs=1, space="PSUM") as ps:

            ones_col = sb.tile([128, 1], F32)
            ones_row = sb.tile([1, 128], F32)
            eps_col = sb.tile([1, 1], F32)
            nc.vector.memset(ones_col[:], 1.0)
            nc.vector.memset(ones_row[:], 1.0)
            nc.vector.memset(eps_col[:], EPS)

            x_cur = sb.tile([128, KC, TR], F32)
            nc.sync.dma_start(x_cur[:], x0_d.ap()[:])

            xh = sb.tile([128, KC, TR], F16)
            xl = sb.tile([128, KC, TR], F16)
            aga = sb.tile([128, 6, TR // 2], F32)
            agb = sb.tile([128, 6, TR // 2], F32)
            wg_buf = sb.tile([128, T, 6, R], F32)
            s_buf = sb.tile([128, T, 6, R], F16)
            kv_buf = sb.tile([128, T, KC, R], F16)
            h_buf = sb.tile([128, T, KC, R], F32)
            hh16 = sb.tile([128, T, KC, R], F16)
            hl16 = sb.tile([128, T, KC, R], F16)
            rhh = sb.tile([128, T, KC, R], F16)
            rhl = sb.tile([128, T, KC, R], F16)
            at_buf = sb.tile([128, KC, TR], F32)
            u_buf = sb.tile([128, KC, TR], F32)
            sq_buf = sb.tile([128, KC, TR], F32)
            x1_buf = sb.tile([128, KC, TR], F32)
            x1h = sb.tile([128, KC, TR], F16)
            x1l = sb.tile([128, KC, TR], F16)
            a1a = sb.tile([128, FC, TR // 2], F32)
            a1b = sb.tile([128, FC, TR // 2], F32)
            w1_buf = sb.tile([128, T, FC, R], F32)
            s1_buf = sb.tile([128, T, FC, R], F16)
            a2a = sb.tile([128, KC, TR // 2], F32)
            a2b = sb.tile([128, KC, TR // 2], F32)
            w2_buf = sb.tile([128, T, KC, R], F32)
            s2_buf = sb.tile([128, T, KC, R], F16)
            zg = sb.tile([128, 6, R], F32)
            zh = sb.tile([128, KC, R], F32)
            z1 = sb.tile([128, FC, R], F32)
            nc.vector.memset(zg[:], 0.0)
            nc.vector.memset(zh[:], 0.0)
            nc.vector.memset(z1[:], 0.0)

            wl16 = [wp.tile([128, W16], F16, tag="w16", name=f"w16_{i}")
                    for i in range(L)]
            wl8 = [wp.tile([128, 2, NB8, 2, 128], F8E4, tag="w8",
                           name=f"w8_{i}") for i in range(L)]
            wl32 = [wp.tile([128, WS], F32, tag="w32", name=f"w32_{i}")
                    for i in range(L)]
            for l in range(L):
                nc.sync.dma_start(wl16[l][:], w16_d.ap()[l])
                nc.sync.dma_start(wl8[l][:], w8_d.ap()[l])
                nc.sync.dma_start(wl32[l][:], w32_d.ap()[l])

            xB8 = sb.tile([128, KC, TR], F8E4)
            xC8 = sb.tile([128, KC, TR], F8E4)

            def tile16(wl, base, idx):
                off = base + idx * 128
                return wl[:, off:off + 128]

            for l in range(L):
                w6, w6_8, w2c = wl16[l], wl8[l], wl32[l]

                if l == 0:
                    # layer 0: x is 0/1 spikes (xl == 0 exactly)
                    nc.vector.tensor_copy(out=xh[:], in_=x_cur[:])
                    nc.scalar.activation(xC8[:], xh[:], AF.Identity,
                                         bias=0.0, scale=1.0)

                # --- gates: xh@Wh (fp16, x2^14) + fp8 DoubleRow corrections
                #     (x-lo)@Wh and xh@(W-lo), K=256 per DR pass ---
                HT = TR // 2
                for half, agx in ((0, aga), (1, agb)):
                    sl = slice(half * HT, (half + 1) * HT)
                    for g in range(3):
                        for hf in range(KC):
                            bank = g * KC + hf
                            passes = [
                                (tile16(w6, GH_OFF, bank * KC + kc),
                                 xh[:, kc, sl]) for kc in range(KC)]
                            if l > 0:
                                passes.append((w6_8[:, 0, bank],
                                               xB8[:, :, sl], DR))
                            passes.append((w6_8[:, 1, bank],
                                           xC8[:, :, sl], DR))
                            _mm16(nc, ps, passes, agx[:, bank, :],
                                  name=f"g{half}{bank}", free=HT,
                                  scale=2.0 ** -14)

                # --- gate LIF scan; per half: spikes, kv, h-recurrence,
                #     rh (as hi/lo via h split: r in {0,1}), Wo matmuls ---
                def ag_src(t):
                    agx = aga if t < 8 else agb
                    tt = t % 8
                    return agx[:, :, tt * R:(tt + 1) * R]

                for t in range(T):
                    nc.vector._custom_dve(
                        lif, out=wg_buf[:, t],
                        in0=(zg[:] if t == 0 else wg_buf[:, t - 1]),
                        in1=ag_src(t), s0=0.5)
                    if t == 7 or t == 15:
                        half = 0 if t == 7 else 1
                        hh = slice(t - 7, t + 1)
                        nc.vector.tensor_scalar(
                            out=s_buf[:, hh], in0=wg_buf[:, hh], scalar1=1.0,
                            scalar2=None, op0=OP.is_ge)
                        nc.vector.tensor_mul(
                            out=kv_buf[:, hh], in0=s_buf[:, hh, 2:4, :],
                            in1=s_buf[:, hh, 4:6, :])
                        for th in range(t - 7, t + 1):
                            nc.vector.scalar_tensor_tensor(
                                out=h_buf[:, th],
                                in0=(zh[:] if th == 0 else h_buf[:, th - 1]),
                                scalar=0.9, in1=kv_buf[:, th],
                                op0=OP.mult, op1=OP.add)
                        # h hi/lo split; rh_hi = r*h_hi, rh_lo = r*h_lo
                        # (exact: r is 0/1)
                        nc.vector.tensor_copy(out=hh16[:, hh], in_=h_buf[:, hh])
                        nc.vector.tensor_sub(out=hl16[:, hh],
                                             in0=h_buf[:, hh], in1=hh16[:, hh])
                        nc.vector.tensor_mul(out=rhh[:, hh],
                                             in0=s_buf[:, hh, 0:2, :],
                                             in1=hh16[:, hh])
                        nc.vector.tensor_mul(out=rhl[:, hh],
                                             in0=s_buf[:, hh, 0:2, :],
                                             in1=hl16[:, hh])
                        for hf in range(KC):
                            passes = []
                            for kc in range(KC):
                                wh = tile16(w6, WOH_OFF, hf * KC + kc)
                                passes += [(wh, rhh[:, hh, kc, :]),
                                           (wh, rhl[:, hh, kc, :])]
                            _mm16(nc, ps, passes,
                                  at_buf[:, hf, half * HT:(half + 1) * HT],
                                  name=f"wo{half}{hf}", free=HT)

                # --- LN1(x + attn) -> x1 and FFN mm1, pipelined per half ---
                for half, a1x in ((0, a1a), (1, a1b)):
                    sl = slice(half * HT, (half + 1) * HT)
                    for kc in range(KC):
                        nc.vector.tensor_add(out=u_buf[:, kc, sl],
                                             in0=x_cur[:, kc, sl],
                                             in1=at_buf[:, kc, sl])
                    _layer_norm(
                        nc, ps, sb, u_buf, sq_buf,
                        lambda kc: w2c[:, LN_OFF + kc:LN_OFF + kc + 1],
                        lambda kc: w2c[:, LN_OFF + 2 + kc:LN_OFF + 2 + kc + 1],
                        lambda kc: x1_buf[:, kc, sl],
                        ones_col, ones_row, eps_col, sl, HT, identity=ln_id)
                    nc.vector.tensor_copy(out=x1h[:, :, sl], in_=x1_buf[:, :, sl])
                    nc.vector.tensor_sub(out=x1l[:, :, sl], in0=x1_buf[:, :, sl],
                                         in1=x1h[:, :, sl])
                    nc.scalar.activation(xB8[:, :, sl], x1l[:, :, sl],
                                         AF.Identity, bias=0.0, scale=2.0 ** 11)
                    nc.scalar.activation(xC8[:, :, sl], x1h[:, :, sl],
                                         AF.Identity, bias=0.0, scale=1.0)
                    for mf in range(FC):
                        passes = [(tile16(w6, W1H_OFF, mf * KC + kc),
                                   x1h[:, kc, sl]) for kc in range(KC)]
                        passes.append((w6_8[:, 0, 6 + mf], xB8[:, :, sl], DR))
                        passes.append((w6_8[:, 1, 6 + mf], xC8[:, :, sl], DR))
                        _mm16(nc, ps, passes, a1x[:, mf, :],
                              bias=w2c[:, B1_OFF + mf:B1_OFF + mf + 1],
                              name=f"f{half}{mf}", free=HT, scale=2.0 ** -14)

                # --- LIF1, spikes per half ---
                def a1_src(t):
                    a1x = a1a if t < 8 else a1b
                    tt = t % 8
                    return a1x[:, :, tt * R:(tt + 1) * R]

                for t in range(T):
                    nc.vector._custom_dve(
                        lif, out=w1_buf[:, t],
                        in0=(z1[:] if t == 0 else w1_buf[:, t - 1]),
                        in1=a1_src(t), s0=0.5)
                    if t == 7 or t == 15:
                        hh = slice(t - 7, t + 1)
                        nc.vector.tensor_scalar(
                            out=s1_buf[:, hh], in0=w1_buf[:, hh], scalar1=1.0,
                            scalar2=None, op0=OP.is_ge)

                # --- mm2 (+b2): s1 exact fp16, 2 passes per K chunk, T-split ---
                for half, a2x in ((0, a2a), (1, a2b)):
                    tsl = slice(half * 8, (half + 1) * 8)
                    for mh in range(KC):
                        passes = [
                            (tile16(w6, W2H_OFF, mh * FC + kc8),
                             s1_buf[:, tsl, kc8, :]) for kc8 in range(FC)]
                        _mm16(nc, ps, passes, a2x[:, mh, :],
                              bias=w2c[:, B2_OFF + mh:B2_OFF + mh + 1],
                              name=f"m2{half}{mh}", free=HT)

                # --- LIF2, spikes per half ---
                def a2_src(t):
                    a2x = a2a if t < 8 else a2b
                    tt = t % 8
                    return a2x[:, :, tt * R:(tt + 1) * R]

                for t in range(T):
                    nc.vector._custom_dve(
                        lif, out=w2_buf[:, t],
                        in0=(zh[:] if t == 0 else w2_buf[:, t - 1]),
                        in1=a2_src(t), s0=0.5)
                    if t == 7 or t == 15:
                        hh = slice(t - 7, t + 1)
                        nc.vector.tensor_scalar(
                            out=s2_buf[:, hh], in0=w2_buf[:, hh], scalar1=1.0,
                            scalar2=None, op0=OP.is_ge)

                # --- LN2(x1 + s2) -> x_cur, per half ---
                for half in (0, 1):
                    sl = slice(half * HT, (half + 1) * HT)
                    tsl = slice(half * 8, (half + 1) * 8)
                    for kc in range(KC):
                        nc.vector.tensor_add(out=u_buf[:, kc, sl],
                                             in0=x1_buf[:, kc, sl],
                                             in1=s2_buf[:, tsl, kc, :])
                    _layer_norm(
                        nc, ps, sb, u_buf, sq_buf,
                        lambda kc: w2c[:, LN_OFF + 4 + kc:LN_OFF + 4 + kc + 1],
                        lambda kc: w2c[:, LN_OFF + 6 + kc:LN_OFF + 6 + kc + 1],
                        lambda kc: x_cur[:, kc, sl],
                        ones_col, ones_row, eps_col, sl, HT, identity=ln_id)
                    if l + 1 < L:
                        nc.vector.tensor_copy(out=xh[:, :, sl],
                                              in_=x_cur[:, :, sl])
                        nc.vector.tensor_sub(out=xl[:, :, sl],
                                             in0=x_cur[:, :, sl],
                                             in1=xh[:, :, sl])
                        nc.scalar.activation(xB8[:, :, sl], xl[:, :, sl],
                                             AF.Identity, bias=0.0,
                                             scale=2.0 ** 11)
                        nc.scalar.activation(xC8[:, :, sl], xh[:, :, sl],
                                             AF.Identity, bias=0.0, scale=1.0)

            nc.sync.dma_start(h_d.ap()[:], x_cur[:])
    nc.compile()
    return nc


def build_head():
    """Head v2: flipped matmul — h-tiles stationary, Wout streams on the
    free dim. Output layout [tn-rows on partitions, vocab on free].

    Per tn-block (t, n-half): logits*2^14 accumulate in PSUM from 4 passes:
      A (fp16):  hh @ (fp16(W0)*2^14), 2 K-chunks
      B (fp8 DoubleRow, K=256 in 1 pass): e4m3(hl*2^11) @ e4m3(Wh*2^3)
      C (fp8 DoubleRow):                  e4m3(hh)      @ e4m3(Wl*2^14)
    ACT drains with scale 2^-18 (the extra /16 feeds the V = w/16 state
    encoding). The scan is ONE giant fused LIF+count DVE op per (t, n-half)
    on [128 x 4096]: state U = V + count/2 in a single fp32 (see
    _get_lifcnt_op). A final flush step (a = 0) counts the last spike;
    the host reads the count as rint(2U)."""
    lifcnt = _get_lifcnt_op()
    nc = bacc.Bacc("TRN2", target_bir_lowering=False)
    hh_d = nc.dram_tensor("hTh", [128, KC, TN], F16, kind="ExternalInput")
    hb_d = nc.dram_tensor("hB8", [128, KC, TN], F8E4, kind="ExternalInput")
    hc_d = nc.dram_tensor("hC8", [128, KC, TN], F8E4, kind="ExternalInput")
    ws_d = nc.dram_tensor("wS", [128, KC, VSH], F16, kind="ExternalInput")
    wb_d = nc.dram_tensor("wB8", [128, KC, VSH], F8E4, kind="ExternalInput")
    wc_d = nc.dram_tensor("wC8", [128, KC, VSH], F8E4, kind="ExternalInput")
    o_d = nc.dram_tensor("out_nh", [2, 128, VSH], F32, kind="ExternalOutput")

    VB = VSH // 512  # 8 psum-bank columns
    with tile.TileContext(nc) as tc:
        with tc.tile_pool(name="sb", bufs=1) as sb, \
             tc.tile_pool(name="ps", bufs=1, space="PSUM") as ps:

            hh = sb.tile([128, KC, TN], F16)
            hb = sb.tile([128, KC, TN], F8E4)
            hc = sb.tile([128, KC, TN], F8E4)
            ws = sb.tile([128, KC, VSH], F16)
            wb = sb.tile([128, KC, VSH], F8E4)
            wc = sb.tile([128, KC, VSH], F8E4)
            # interleave DMAs so tile-0 operands land first
            QT, QV = TN // 4, VSH // 4
            for q in range(4):
                for kc in range(KC):
                    nc.sync.dma_start(ws[:, kc, q * QV:(q + 1) * QV],
                                      ws_d.ap()[:, kc, q * QV:(q + 1) * QV])
                    nc.sync.dma_start(hh[:, kc, q * QT:(q + 1) * QT],
                                      hh_d.ap()[:, kc, q * QT:(q + 1) * QT])
                    nc.sync.dma_start(wb[:, kc, q * QV:(q + 1) * QV],
                                      wb_d.ap()[:, kc, q * QV:(q + 1) * QV])
                    nc.sync.dma_start(wc[:, kc, q * QV:(q + 1) * QV],
                                      wc_d.ap()[:, kc, q * QV:(q + 1) * QV])
                    nc.sync.dma_start(hb[:, kc, q * QT:(q + 1) * QT],
                                      hb_d.ap()[:, kc, q * QT:(q + 1) * QT])
                    nc.sync.dma_start(hc[:, kc, q * QT:(q + 1) * QT],
                                      hc_d.ap()[:, kc, q * QT:(q + 1) * QT])

            w_st = [sb.tile([128, 32, 128], F32, name=f"wst{nh}")
                    for nh in range(2)]
            zeros = sb.tile([128, 32, 128], F32, name="zeros")
            for nh in range(2):
                nc.vector.memset(w_st[nh][:], 0.0)
            nc.vector.memset(zeros[:], 0.0)

            a_ring = [sb.tile([128, 32, 128], F32, name=f"a{k}")
                      for k in range(4)]

            for t in range(T):
                for nh in range(2):
                    tb = t * 2 + nh
                    slot = a_ring[tb % 4]
                    for vh in range(2):
                        bank = ps.tile([128, 2048], F32, tag="mm",
                                       name=f"mm{tb}_{vh}", bufs=2)
                        hsl = slice(tb * 128, (tb + 1) * 128)
                        # A: fp16 hi passes (pass-outer, bank-inner: one
                        # weight load streams 4 x 512)
                        for kc in range(KC):
                            lhsT = hh[:, kc, hsl]
                            for b in range(4):
                                off = vh * 2048 + b * 512
                                nc.tensor.matmul(
                                    bank[:, b * 512:(b + 1) * 512], lhsT,
                                    ws[:, kc, off:off + 512],
                                    start=(kc == 0), stop=False)
                        # B, C: fp8 DoubleRow, K=256 in one pass each
                        for i, (hsrc, wsrc) in enumerate(((hb, wb), (hc, wc))):
                            lhsT = hsrc[:, :, hsl]
                            for b in range(4):
                                off = vh * 2048 + b * 512
                                nc.tensor.matmul(
                                    bank[:, b * 512:(b + 1) * 512], lhsT,
                                    wsrc[:, :, off:off + 512],
                                    start=False, stop=(i == 1 and b == 3),
                                    perf_mode=DR)
                        nc.scalar.activation(
                            slot[:, vh * 16:(vh + 1) * 16, :], bank[:],
                            AF.Identity, bias=0.0, scale=2.0 ** -18)
                    # fused LIF + count scan step, in-place state
                    nc.vector._custom_dve(
                        lifcnt, out=w_st[nh][:], in0=w_st[nh][:],
                        in1=slot[:], s0=0.25, s1=1.5 * 2.0 ** 22)

            for nh in range(2):
                # flush: one extra step (a=0) counts the final state's spike
                nc.vector._custom_dve(
                    lifcnt, out=w_st[nh][:], in0=w_st[nh][:],
                    in1=zeros[:], s0=0.25, s1=1.5 * 2.0 ** 22)
                nc.sync.dma_start(o_d.ap()[nh], w_st[nh][:])
    nc.compile()
    return nc


_CACHE = {}
TRACE = False
LAST = {}


def _run(nc, in_maps, key):
    import tempfile

    if TRACE:
        td = tempfile.mkdtemp(prefix=f"bkt_{key}_")
        res = run_bass_kernel_spmd(nc, in_maps, core_ids=list(range(NCORE)),
                                   trace=True, tmpdir=td)
        LAST[key] = (res, td)
        return res
    return run_bass_kernel_spmd(nc, in_maps, core_ids=list(range(NCORE)))


def _get_programs(ln_id):
    key = f"blocks{ln_id}"
    if key not in _CACHE:
        _CACHE[key] = build_blocks(ln_id=ln_id)
    if "head" not in _CACHE:
        _CACHE["head"] = build_head()
    return _CACHE[key], _CACHE["head"]


def _pack_weights(Wr, Wk, Wv, Wo, W1, b1, W2, b2, g1, be1, g2, be2):
    import ml_dtypes
    e4t = ml_dtypes.float8_e4m3
    w16 = np.zeros((L, 128, W16), np.float16)
    w8 = np.zeros((L, 128, 2, NB8, 2, 128), e4t)
    w32 = np.zeros((L, 128, WS), np.float32)
    for l in range(L):
        his = []

        def add(mat):  # [K, M] fp32 -> fp16 hi (+ lo for Wo/W2)
            hi, lo = _split16(mat)
            his.append(hi)
            return lo

        # gates + W1: hi fp16 pre-scaled 2^14; B/C corrections fp8 DR tiles
        gh = []
        for bank in range(NB8):
            for kc in range(KC):
                if bank < 6:
                    g, hf = divmod(bank, KC)
                    Wg = (Wr, Wk, Wv)[g]
                    blk = 0.5 * Wg[l][kc * 128:(kc + 1) * 128,
                                      hf * 128:(hf + 1) * 128]
                else:
                    mf = bank - 6
                    blk = 0.5 * W1[l][kc * 128:(kc + 1) * 128,
                                      mf * 128:(mf + 1) * 128]
                hi = blk.astype(np.float16)
                lo = blk - hi.astype(np.float32)
                gh.append((hi.astype(np.float32) * 2.0 ** 14)
                          .astype(np.float16))
                w8[l, :, 0, bank, kc, :] = (hi.astype(np.float32) * 2.0 ** 3
                                            ).astype(e4t)
                w8[l, :, 1, bank, kc, :] = (lo * 2.0 ** 14).astype(e4t)
        ghs = np.concatenate(gh[:12], axis=1)     # gates hi
        w1h = np.concatenate(gh[12:], axis=1)     # W1 hi
        his = []
        los = []
        for hf in range(KC):
            for kc in range(KC):
                los.append(add(Wo[l][kc * 128:(kc + 1) * 128,
                                     hf * 128:(hf + 1) * 128]))
        woh = np.concatenate(his, axis=1)
        wol = np.concatenate([x.astype(np.float16) for x in los], axis=1)
        his, los = [], []
        for mh in range(KC):
            for kc8 in range(FC):
                los.append(add(0.5 * W2[l][kc8 * 128:(kc8 + 1) * 128,
                                           mh * 128:(mh + 1) * 128]))
        w2h = np.concatenate(his, axis=1)
        w2l = np.concatenate([x.astype(np.float16) for x in los], axis=1)
        w16[l] = np.concatenate([ghs, woh, wol, w1h, w2h, w2l], axis=1)
        w32[l] = np.concatenate([
            0.5 * b1[l].reshape(FC, 128).T,
            0.5 * b2[l].reshape(KC, 128).T,
            g1[l].reshape(KC, 128).T, be1[l].reshape(KC, 128).T,
            g2[l].reshape(KC, 128).T, be2[l].reshape(KC, 128).T,
        ], axis=1)
    return (np.ascontiguousarray(w16), np.ascontiguousarray(w8),
            np.ascontiguousarray(w32))


def kernel(input_ids, token_embedding, pos_embedding, noise, unif,
           Wr, Wk, Wv, Wo, W1, b1, W2, b2, ln1_g, ln1_b, ln2_g, ln2_b,
           Wout, bout):
    input_ids = np.asarray(input_ids)
    f32 = lambda a: np.asarray(a, dtype=np.float32)
    token_embedding, pos_embedding, noise, unif = map(
        f32, (token_embedding, pos_embedding, noise, unif))
    Wr, Wk, Wv, Wo, W1, b1, W2, b2 = map(f32, (Wr, Wk, Wv, Wo, W1, b1, W2, b2))
    ln1_g, ln1_b, ln2_g, ln2_b, Wout, bout = map(
        f32, (ln1_g, ln1_b, ln2_g, ln2_b, Wout, bout))

    ln_id = bool((ln1_g == 1).all() and (ln1_b == 0).all()
                 and (ln2_g == 1).all() and (ln2_b == 0).all())
    nc_blocks, nc_head = _get_programs(ln_id)

    spikes = _encode_spikes(input_ids, token_embedding, pos_embedding, noise, unif)
    sp = spikes.reshape(T, NCORE, R, KC, 128)          # (t, core, r, kc, p)
    x0 = np.ascontiguousarray(sp.transpose(1, 4, 3, 0, 2)).reshape(NCORE, 128, KC, TR)
    w16, w8, w32 = _pack_weights(Wr, Wk, Wv, Wo, W1, b1, W2, b2,
                                 ln1_g, ln1_b, ln2_g, ln2_b)
    in1 = [{"x0": x0[c], "w16": w16, "w8": w8, "w32": w32}
           for c in range(NCORE)]
    res1 = _run(nc_blocks, in1, "blocks")
    ho = np.stack([res1.results[c]["h_out"].reshape(128, KC, T, R)
                   for c in range(NCORE)])
    hT = np.ascontiguousarray(ho.transpose(1, 2, 3, 0, 4)).reshape(128, KC, TN)
    import ml_dtypes
    e4 = ml_dtypes.float8_e4m3
    hTh16 = hT.astype(np.float16)
    hTl = hT - hTh16.astype(np.float32)
    hB8 = np.ascontiguousarray((hTl * 2.0 ** 11).astype(e4))
    hC8 = np.ascontiguousarray(hTh16.astype(np.float32).astype(e4))
    hTh16 = np.ascontiguousarray(hTh16)

    assert not np.any(bout), "head kernel assumes bout == 0 (spec fill=zeros)"
    Wp = np.zeros((D, VPAD), np.float32)
    Wp[:, :V] = 0.5 * Wout
    Wph16 = Wp.astype(np.float16)
    Wpl = Wp - Wph16.astype(np.float32)
    WSc = (Wph16.astype(np.float32) * 2.0 ** 14).astype(np.float16)
    WB8 = (Wph16.astype(np.float32) * 2.0 ** 3).astype(e4)
    WC8 = (Wpl * 2.0 ** 14).astype(e4)

    def shard(Wx, c):
        w = Wx[:, c * VSH:(c + 1) * VSH].reshape(KC, 128, VSH)
        return np.ascontiguousarray(w.transpose(1, 0, 2))
    in2 = [{"hTh": hTh16, "hB8": hB8, "hC8": hC8,
            "wS": shard(WSc, c), "wB8": shard(WB8, c), "wC8": shard(WC8, c)}
           for c in range(NCORE)]
    res2 = _run(nc_head, in2, "head")
    # out_nh[nh, p, v] holds U = V + count/2: count = rint(2U).
    # row n = nh*128 + p, vocab col = c*VSH + v
    out_sh = np.stack([res2.results[c]["out_nh"] for c in range(NCORE)])
    out = np.empty((N, VPAD), np.float32)
    for c in range(NCORE):
        for nh in range(2):
            out[nh * 128:(nh + 1) * 128, c * VSH:(c + 1) * VSH] = \
                np.rint(2.0 * out_sh[c, nh].astype(np.float64)).reshape(128, VSH)
    out = out[:, :V].reshape(B, S, V).astype(np.float32)
    return out



# revision 31
# speedup vs baseline: 1.0510x; 1.0028x over previous
"""Trainium2 Bass kernel for nn_AdvancedSpikingChatModel.

Model: spike-encode embeddings -> 6 spiking-transformer blocks (LIF gates +
decaying linear-attention recurrence over T=16) -> LIF output head with
spike-count accumulation over V=32000 vocab.

Strategy (8 NeuronCores, SPMD, two launches):
  Launch 1 (blocks): data-parallel over the 256 folded (b,s) rows, 32/core.
    Features on partitions, (t, row) on the free dim; weights stationary.
  Launch 2 (head): vocab-parallel, 4096 padded vocab cols/core. Flipped
    matmul: h-tiles stationary, Wout streams on the free dim, so the output
    lands as [tn-rows x vocab] and the LIF scan runs as 32 giant DVE ops.

Precision: main matmul passes are fp16 hi (x2^14 PSUM scale); the hi/lo
cross terms (x_lo@W_hi, x_hi@W_lo) run as fp8e4m3 DoubleRow passes (K=256
per pass at 0.5 cyc/row), scale-aligned so all passes share one PSUM bank
(ACT drain rescales by 2^-14). Residual error ~2^-15, measured end-to-end
rel err 6.4e-3 vs the 2e-2 gate. Wo keeps a 2-pass fp16 scheme; W2's input
(0/1 spikes) is exact fp16, single pass.

Blocks LIF: w' = 0.5*(min(w,1) - (w>=1)) + a, one custom DVE op per step;
spikes recovered with batched is_ge. Head LIF: fused LIF+count custom op —
state U = V + count/2 in one fp32 (V = w/16), spike threshold/decay/count
and the round-to-half magic all inside one 8-stage DVE pass; the host
decodes counts as rint(2U). See _get_lifcnt_op.
"""

import numpy as np

import concourse.mybir as mybir
import concourse.tile as tile
from concourse import bacc
from concourse.bass_utils import run_bass_kernel_spmd

F32 = mybir.dt.float32
F16 = mybir.dt.float16
F8E4 = mybir.dt.float8e4
OP = mybir.AluOpType
AF = mybir.ActivationFunctionType
DR = mybir.MatmulPerfMode.DoubleRow

B, S, D, T, L, F, V = 2, 128, 256, 16, 6, 1024, 32000
N = B * S
NCORE = 8
R = N // NCORE       # 32 rows/core in launch 1
TR = T * R           # 512
KC = D // 128
FC = F // 128
VPAD = 32768
VSH = VPAD // NCORE  # 4096
VCH = VSH // 128     # 32 chunks
TN = T * N           # 4096
EPS = 1e-5

# fp16 weight slab offsets (fp16 words per partition, per layer).
# Gates/W1 hi tiles are pre-scaled by 2^14 (PSUM scale shared with the
# fp8 DoubleRow correction passes; drained with ACT scale 2^-14).
GH_OFF = 0
WOH_OFF = GH_OFF + 12 * 128
WOL_OFF = WOH_OFF + 4 * 128
W1H_OFF = WOL_OFF + 4 * 128
W2H_OFF = W1H_OFF + 16 * 128
W2L_OFF = W2H_OFF + 16 * 128
W16 = W2L_OFF + 16 * 128
# fp8 slab: [L, 128, 2(B/C), 14 banks, 2 kc, 128] — banks 0-5 gates, 6-13 W1.
# B = e4m3(Wh * 2^3) pairs with x-lo * 2^11; C = e4m3(Wl * 2^14) with x-hi.
NB8 = 14
# fp32 smalls: b1(8) b2(2) ln(8)
B1_OFF = 0
B2_OFF = 8
LN_OFF = 10
WS = 18

_LIF_OP = None
_LIFCNT_OP = None


def _register_op(name, spec):
    from concourse.dve_ops import (
        DveOp, OPS, _SUB_OPCODE_FOR_NAME, CUSTOM_DVE_SPECS)
    from concourse.dve_spec import lower
    from concourse.dve_uop import DveOpSpec

    if name in _SUB_OPCODE_FOR_NAME:
        return next(o for o in OPS if o.name == name)
    op = DveOp(name, spec, subdim=False, uops_sha={})
    row = 1 + len(OPS)
    OPS.append(op)
    _SUB_OPCODE_FOR_NAME[name] = row
    CUSTOM_DVE_SPECS[name] = spec
    for ver in ("v3",):
        s = DveOpSpec(name=name, opcode=row, uops=lower(spec, ver=ver),
                      rd1_en=True)
        op.uops_sha[ver] = s.sha(ver)
    return op


def _get_lif_op():
    """LIF step as a custom DVE op: out = (min(w,1) - (w>=1))*0.5 + a."""
    global _LIF_OP
    if _LIF_OP is None:
        from concourse.dve_spec import Spec, Src0, Src1, C0, One, minn
        body = (minn(Src0, One) - (Src0 >= One)) * C0 + Src1
        _LIF_OP = _register_op("LIF_STEP_ANT", Spec(
            body=body,
            reference=lambda in0, in1, s0, s1, imm2:
                (np.minimum(in0, 1.0) - (in0 >= 1.0)) * s0 + in1,
        ))
    return _LIF_OP


def _get_lifcnt_op():
    """Fused LIF step + spike count, one DVE pass (8 ALU stages).

    State U = V + A/2 in one fp32: V = w/16 in (-0.25, 0.25) is the membrane,
    A the spike count. s0 = 0.25 (latches derive threshold 1/16 = s0^2 and
    decay 0.5 = s0+s0), s1 = 1.5*2^22 (magic: (U+M)-M rounds U to the
    nearest 0.5 multiple = A/2, exact for |V| < 0.25 both signs).
    Spike branch outputs One, halved to +0.5 == one count unit, V reset 0.
    in1 = a/16 (pre-scaled in the PSUM drain)."""
    global _LIFCNT_OP
    if _LIFCNT_OP is None:
        from concourse.dve_spec import (
            Spec, Src0, Src1, C0, C1, One, Latch, select)
        T16 = Latch(C0 * C0)
        Half = Latch(C0 + C0)
        m1 = Src0 + C1
        r = m1 - C1
        V = Src0 - r
        g = V >= T16
        body = select(g, One, V) * Half + (Src1 + r)

        def ref(in0, in1, s0, s1, imm2):
            f32 = np.float32
            in0 = np.asarray(in0, f32)
            in1 = np.asarray(in1, f32)
            r = (in0 + f32(s1)).astype(f32) - f32(s1)
            V = in0 - r
            g = V >= f32(s0) * f32(s0)
            selv = np.where(g, f32(1.0), V)
            return selv * (f32(s0) + f32(s0)) + (in1 + r)

        _LIFCNT_OP = _register_op("LIF_CNT_ANT", Spec(body=body, reference=ref))
    return _LIFCNT_OP


def _sigmoid(x):
    return 1.0 / (1.0 + np.exp(-x))


def _encode_spikes(input_ids, token_embedding, pos_embedding, noise, unif):
    """Host-side rate coding; (0.7*rate + 0.3*temp > 0.5) == rate exactly."""
    emb = token_embedding[input_ids] + pos_embedding[None, :S]
    p = np.clip(_sigmoid(emb) * 0.8 + 0.1 + noise * 0.05, 0.0, 1.0)
    return (unif < p[None]).astype(np.float32)


def _split16(x):
    hi = x.astype(np.float16)
    lo = (x - hi.astype(np.float32)).astype(np.float16)
    return hi, lo


def _mm16(nc, ps, passes, dst_ap, bias=0.0, name="mmb", free=512, scale=1.0):
    """Accumulate matmul passes into one PSUM bank, ACT-copy(+bias) out.
    A pass is (lhsT, rhs) fp16 or (lhsT, rhs, perf_mode) for fp8 DR."""
    bank = ps.tile([128, free], F32, tag="mm", name=name, bufs=4)
    npass = len(passes)
    for i, p in enumerate(passes):
        pm = p[2] if len(p) > 2 else None
        nc.tensor.matmul(bank[:], p[0], p[1],
                         start=(i == 0), stop=(i == npass - 1), perf_mode=pm)
    nc.scalar.activation(dst_ap, bank[:], AF.Identity, bias=bias, scale=scale)


def _w_scan(nc, lif, w_buf, z0, a_fn, nt=T, sliced=False):
    """w_t = (min(w_{t-1},1) - (w_{t-1}>=1))*0.5 + a_t via the custom op.
    in1 must keep >=2 free dims (STT encoding; the TTSS form runs ~10x slower)."""
    for t in range(nt):
        if sliced:
            out = w_buf[:, t:t + 1, :]
            in0 = z0[:] if t == 0 else w_buf[:, t - 1:t, :]
        else:
            out = w_buf[:, t]
            in0 = z0[:] if t == 0 else w_buf[:, t - 1]
        nc.vector._custom_dve(lif, out=out, in0=in0, in1=a_fn(t), s0=0.5)


def _layer_norm(nc, ps, sb, u, sq_buf, gamma_col, beta_col, out_fn,
                ones_col, ones_row, eps_col, csl, W, identity=False):
    """LN over features (partitions x KC chunks) on a column slice csl of
    width W. u: [128, KC, TR] fp32; out_fn(kc) -> dst AP for that slice.
    identity=True skips the gamma/beta affine (gamma==1, beta==0)."""
    for kc in range(KC):
        nc.scalar.activation(sq_buf[:, kc, csl], u[:, kc, csl], AF.Square)
    ps_m = ps.tile([1, W], F32, tag="st", name="ps_m", bufs=2)
    ps_q = ps.tile([1, W], F32, tag="st", name="ps_q", bufs=2)
    for kc in range(KC):
        nc.tensor.matmul(ps_m[:], ones_col[:], u[:, kc, csl],
                         start=(kc == 0), stop=(kc == KC - 1))
    for kc in range(KC):
        nc.tensor.matmul(ps_q[:], ones_col[:], sq_buf[:, kc, csl],
                         start=(kc == 0), stop=(kc == KC - 1))
    m_sb = sb.tile([1, W], F32, name="m_sb", tag="m_sb", bufs=2)
    q_sb = sb.tile([1, W], F32, name="q_sb", tag="q_sb", bufs=2)
    nc.scalar.mul(m_sb[:], ps_m[:], 1.0 / D)
    nc.scalar.mul(q_sb[:], ps_q[:], 1.0 / D)
    ve = sb.tile([1, W], F32, name="ve", tag="ve", bufs=2)
    nc.vector.tensor_mul(out=ve[:], in0=m_sb[:], in1=m_sb[:])
    nc.vector.tensor_sub(out=ve[:], in0=q_sb[:], in1=ve[:])
    # rstd = 1/sqrt(var+eps): ACT sqrt (eps via bias) + fast reciprocal
    r0 = sb.tile([1, W], F32, name="r0", tag="r0", bufs=2)
    nc.scalar.activation(r0[:], ve[:], AF.Sqrt, bias=eps_col[:])
    nc.vector.reciprocal_approx_fast(r0[:], r0[:])
    pb_m = ps.tile([128, W], F32, tag="bc", name="pb_m", bufs=2)
    pb_r = ps.tile([128, W], F32, tag="bc", name="pb_r", bufs=2)
    nc.tensor.matmul(pb_m[:], ones_row[:], m_sb[:], start=True, stop=True)
    nc.tensor.matmul(pb_r[:], ones_row[:], r0[:], start=True, stop=True)
    for kc in range(KC):
        o = out_fn(kc)
        nc.vector.tensor_sub(out=o, in0=u[:, kc, csl], in1=pb_m[:])
        nc.vector.tensor_mul(out=o, in0=o, in1=pb_r[:])
        if not identity:
            nc.vector.tensor_scalar(out=o, in0=o, scalar1=gamma_col(kc),
                                    scalar2=beta_col(kc), op0=OP.mult,
                                    op1=OP.add)


def build_blocks(ln_id=True):
    lif = _get_lif_op()
    nc = bacc.Bacc("TRN2", target_bir_lowering=False)
    x0_d = nc.dram_tensor("x0", [128, KC, TR], F32, kind="ExternalInput")
    w16_d = nc.dram_tensor("w16", [L, 128, W16], F16, kind="ExternalInput")
    w8_d = nc.dram_tensor("w8", [L, 128, 2, NB8, 2, 128], F8E4,
                          kind="ExternalInput")
    w32_d = nc.dram_tensor("w32", [L, 128, WS], F32, kind="ExternalInput")
    h_d = nc.dram_tensor("h_out", [128, KC, TR], F32, kind="ExternalOutput")

    with tile.TileContext(nc) as tc:
        with tc.tile_pool(name="wp", bufs=2) as wp, \
             tc.tile_pool(name="sb", bufs=1) as sb, \
             tc.tile_pool(name="ps", bufs=1, space="PSUM") as ps:

            ones_col = sb.tile([128, 1], F32)
            ones_row = sb.tile([1, 128], F32)
            eps_col = sb.tile([1, 1], F32)
            nc.vector.memset(ones_col[:], 1.0)
            nc.vector.memset(ones_row[:], 1.0)
            nc.vector.memset(eps_col[:], EPS)

            x_cur = sb.tile([128, KC, TR], F32)
            nc.sync.dma_start(x_cur[:], x0_d.ap()[:])

            xh = sb.tile([128, KC, TR], F16)
            xl = sb.tile([128, KC, TR], F16)
            aga = sb.tile([128, 6, TR // 2], F32)
            agb = sb.tile([128, 6, TR // 2], F32)
            wg_buf = sb.tile([128, T, 6, R], F32)
            s_buf = sb.tile([128, T, 6, R], F16)
            kv_buf = sb.tile([128, T, KC, R], F16)
            h_buf = sb.tile([128, T, KC, R], F32)
            hh16 = sb.tile([128, T, KC, R], F16)
            hl16 = sb.tile([128, T, KC, R], F16)
            rhh = sb.tile([128, T, KC, R], F16)
            rhl = sb.tile([128, T, KC, R], F16)
            at_buf = sb.tile([128, KC, TR], F32)
            u_buf = sb.tile([128, KC, TR], F32)
            sq_buf = sb.tile([128, KC, TR], F32)
            x1_buf = sb.tile([128, KC, TR], F32)
            x1h = sb.tile([128, KC, TR], F16)
            x1l = sb.tile([128, KC, TR], F16)
            a1a = sb.tile([128, FC, TR // 2], F32)
            a1b = sb.tile([128, FC, TR // 2], F32)
            w1_buf = sb.tile([128, T, FC, R], F32)
            s1_buf = sb.tile([128, T, FC, R], F16)
            a2a = sb.tile([128, KC, TR // 2], F32)
            a2b = sb.tile([128, KC, TR // 2], F32)
            w2_buf = sb.tile([128, T, KC, R], F32)
            s2_buf = sb.tile([128, T, KC, R], F16)
            zg = sb.tile([128, 6, R], F32)
            zh = sb.tile([128, KC, R], F32)
            z1 = sb.tile([128, FC, R], F32)
            nc.vector.memset(zg[:], 0.0)
            nc.vector.memset(zh[:], 0.0)
            nc.vector.memset(z1[:], 0.0)

            wl16 = [wp.tile([128, W16], F16, tag="w16", name=f"w16_{i}")
                    for i in range(L)]
            wl8 = [wp.tile([128, 2, NB8, 2, 128], F8E4, tag="w8",
                           name=f"w8_{i}") for i in range(L)]
            wl32 = [wp.tile([128, WS], F32, tag="w32", name=f"w32_{i}")
                    for i in range(L)]
            for l in range(L):
                nc.sync.dma_start(wl16[l][:], w16_d.ap()[l])
                nc.sync.dma_start(wl8[l][:], w8_d.ap()[l])
                nc.sync.dma_start(wl32[l][:], w32_d.ap()[l])

            xB8 = sb.tile([128, KC, TR], F8E4)
            xC8 = sb.tile([128, KC, TR], F8E4)

            def tile16(wl, base, idx):
                off = base + idx * 128
                return wl[:, off:off + 128]

            for l in range(L):
                w6, w6_8, w2c = wl16[l], wl8[l], wl32[l]

                if l == 0:
                    # layer 0: x is 0/1 spikes (xl == 0 exactly)
                    nc.vector.tensor_copy(out=xh[:], in_=x_cur[:])
                    nc.scalar.activation(xC8[:], xh[:], AF.Identity,
                                         bias=0.0, scale=1.0)

                # --- gates: xh@Wh (fp16, x2^14) + fp8 DoubleRow corrections
                #     (x-lo)@Wh and xh@(W-lo), K=256 per DR pass ---
                HT = TR // 2
                for half, agx in ((0, aga), (1, agb)):
                    sl = slice(half * HT, (half + 1) * HT)
                    for g in range(3):
                        for hf in range(KC):
                            bank = g * KC + hf
                            passes = [
                                (tile16(w6, GH_OFF, bank * KC + kc),
                                 xh[:, kc, sl]) for kc in range(KC)]
                            if l > 0:
                                passes.append((w6_8[:, 0, bank],
                                               xB8[:, :, sl], DR))
                            passes.append((w6_8[:, 1, bank],
                                           xC8[:, :, sl], DR))
                            _mm16(nc, ps, passes, agx[:, bank, :],
                                  name=f"g{half}{bank}", free=HT,
                                  scale=2.0 ** -14)

                # --- gate LIF scan; per half: spikes, kv, h-recurrence,
                #     rh (as hi/lo via h split: r in {0,1}), Wo matmuls ---
                def ag_src(t):
                    agx = aga if t < 8 else agb
                    tt = t % 8
                    return agx[:, :, tt * R:(tt + 1) * R]

                for t in range(T):
                    nc.vector._custom_dve(
                        lif, out=wg_buf[:, t],
                        in0=(zg[:] if t == 0 else wg_buf[:, t - 1]),
                        in1=ag_src(t), s0=0.5)
                    if t == 7 or t == 15:
                        half = 0 if t == 7 else 1
                        hh = slice(t - 7, t + 1)
                        nc.vector.tensor_scalar(
                            out=s_buf[:, hh], in0=wg_buf[:, hh], scalar1=1.0,
                            scalar2=None, op0=OP.is_ge)
                        nc.vector.tensor_mul(
                            out=kv_buf[:, hh], in0=s_buf[:, hh, 2:4, :],
                            in1=s_buf[:, hh, 4:6, :])
                        for th in range(t - 7, t + 1):
                            nc.vector.scalar_tensor_tensor(
                                out=h_buf[:, th],
                                in0=(zh[:] if th == 0 else h_buf[:, th - 1]),
                                scalar=0.9, in1=kv_buf[:, th],
                                op0=OP.mult, op1=OP.add)
                        # h hi/lo split; rh_hi = r*h_hi, rh_lo = r*h_lo
                        # (exact: r is 0/1)
                        nc.vector.tensor_copy(out=hh16[:, hh], in_=h_buf[:, hh])
                        nc.vector.tensor_sub(out=hl16[:, hh],
                                             in0=h_buf[:, hh], in1=hh16[:, hh])
                        nc.vector.tensor_mul(out=rhh[:, hh],
                                             in0=s_buf[:, hh, 0:2, :],
                                             in1=hh16[:, hh])
                        nc.vector.tensor_mul(out=rhl[:, hh],
                                             in0=s_buf[:, hh, 0:2, :],
                                             in1=hl16[:, hh])
                        for hf in range(KC):
                            passes = []
                            for kc in range(KC):
                                wh = tile16(w6, WOH_OFF, hf * KC + kc)
                                passes += [(wh, rhh[:, hh, kc, :]),
                                           (wh, rhl[:, hh, kc, :])]
                            _mm16(nc, ps, passes,
                                  at_buf[:, hf, half * HT:(half + 1) * HT],
                                  name=f"wo{half}{hf}", free=HT)

                # --- LN1(x + attn) -> x1 and FFN mm1, pipelined per half ---
                for half, a1x in ((0, a1a), (1, a1b)):
                    sl = slice(half * HT, (half + 1) * HT)
                    for kc in range(KC):
                        nc.vector.tensor_add(out=u_buf[:, kc, sl],
                                             in0=x_cur[:, kc, sl],
                                             in1=at_buf[:, kc, sl])
                    _layer_norm(
                        nc, ps, sb, u_buf, sq_buf,
                        lambda kc: w2c[:, LN_OFF + kc:LN_OFF + kc + 1],
                        lambda kc: w2c[:, LN_OFF + 2 + kc:LN_OFF + 2 + kc + 1],
                        lambda kc: x1_buf[:, kc, sl],
                        ones_col, ones_row, eps_col, sl, HT, identity=ln_id)
                    nc.vector.tensor_copy(out=x1h[:, :, sl], in_=x1_buf[:, :, sl])
                    nc.vector.tensor_sub(out=x1l[:, :, sl], in0=x1_buf[:, :, sl],
                                         in1=x1h[:, :, sl])
                    nc.scalar.activation(xB8[:, :, sl], x1l[:, :, sl],
                                         AF.Identity, bias=0.0, scale=2.0 ** 11)
                    nc.scalar.activation(xC8[:, :, sl], x1h[:, :, sl],
                                         AF.Identity, bias=0.0, scale=1.0)
                    for mf in range(FC):
                        passes = [(tile16(w6, W1H_OFF, mf * KC + kc),
                                   x1h[:, kc, sl]) for kc in range(KC)]
                        passes.append((w6_8[:, 0, 6 + mf], xB8[:, :, sl], DR))
                        passes.append((w6_8[:, 1, 6 + mf], xC8[:, :, sl], DR))
                        _mm16(nc, ps, passes, a1x[:, mf, :],
                              bias=w2c[:, B1_OFF + mf:B1_OFF + mf + 1],
                              name=f"f{half}{mf}", free=HT, scale=2.0 ** -14)

                # --- LIF1, spikes per half ---
                def a1_src(t):
                    a1x = a1a if t < 8 else a1b
                    tt = t % 8
                    return a1x[:, :, tt * R:(tt + 1) * R]

                for t in range(T):
                    nc.vector._custom_dve(
                        lif, out=w1_buf[:, t],
                        in0=(z1[:] if t == 0 else w1_buf[:, t - 1]),
                        in1=a1_src(t), s0=0.5)
                    if t == 7 or t == 15:
                        hh = slice(t - 7, t + 1)
                        nc.vector.tensor_scalar(
                            out=s1_buf[:, hh], in0=w1_buf[:, hh], scalar1=1.0,
                            scalar2=None, op0=OP.is_ge)

                # --- mm2 (+b2): s1 exact fp16, 2 passes per K chunk, T-split ---
                for half, a2x in ((0, a2a), (1, a2b)):
                    tsl = slice(half * 8, (half + 1) * 8)
                    for mh in range(KC):
                        passes = [
                            (tile16(w6, W2H_OFF, mh * FC + kc8),
                             s1_buf[:, tsl, kc8, :]) for kc8 in range(FC)]
                        _mm16(nc, ps, passes, a2x[:, mh, :],
                              bias=w2c[:, B2_OFF + mh:B2_OFF + mh + 1],
                              name=f"m2{half}{mh}", free=HT)

                # --- LIF2, spikes per half ---
                def a2_src(t):
                    a2x = a2a if t < 8 else a2b
                    tt = t % 8
                    return a2x[:, :, tt * R:(tt + 1) * R]

                for t in range(T):
                    nc.vector._custom_dve(
                        lif, out=w2_buf[:, t],
                        in0=(zh[:] if t == 0 else w2_buf[:, t - 1]),
                        in1=a2_src(t), s0=0.5)
                    if t == 7 or t == 15:
                        hh = slice(t - 7, t + 1)
                        nc.vector.tensor_scalar(
                            out=s2_buf[:, hh], in0=w2_buf[:, hh], scalar1=1.0,
                            scalar2=None, op0=OP.is_ge)

                # --- LN2(x1 + s2) -> x_cur, per half ---
                for half in (0, 1):
                    sl = slice(half * HT, (half + 1) * HT)
                    tsl = slice(half * 8, (half + 1) * 8)
                    for kc in range(KC):
                        nc.vector.tensor_add(out=u_buf[:, kc, sl],
                                             in0=x1_buf[:, kc, sl],
                                             in1=s2_buf[:, tsl, kc, :])
                    _layer_norm(
                        nc, ps, sb, u_buf, sq_buf,
                        lambda kc: w2c[:, LN_OFF + 4 + kc:LN_OFF + 4 + kc + 1],
                        lambda kc: w2c[:, LN_OFF + 6 + kc:LN_OFF + 6 + kc + 1],
                        lambda kc: x_cur[:, kc, sl],
                        ones_col, ones_row, eps_col, sl, HT, identity=ln_id)
                    if l + 1 < L:
                        nc.vector.tensor_copy(out=xh[:, :, sl],
                                              in_=x_cur[:, :, sl])
                        nc.vector.tensor_sub(out=xl[:, :, sl],
                                             in0=x_cur[:, :, sl],
                                             in1=xh[:, :, sl])
                        nc.scalar.activation(xB8[:, :, sl], xl[:, :, sl],
                                             AF.Identity, bias=0.0,
                                             scale=2.0 ** 11)
                        nc.scalar.activation(xC8[:, :, sl], xh[:, :, sl],
                                             AF.Identity, bias=0.0, scale=1.0)

            nc.sync.dma_start(h_d.ap()[:], x_cur[:])
    nc.compile()
    return nc


def build_head():
    """Head v2: flipped matmul — h-tiles stationary, Wout streams on the
    free dim. Output layout [tn-rows on partitions, vocab on free].

    Per tn-block (t, n-half): logits*2^14 accumulate in PSUM from 4 passes:
      A (fp16):  hh @ (fp16(W0)*2^14), 2 K-chunks
      B (fp8 DoubleRow, K=256 in 1 pass): e4m3(hl*2^11) @ e4m3(Wh*2^3)
      C (fp8 DoubleRow):                  e4m3(hh)      @ e4m3(Wl*2^14)
    ACT drains with scale 2^-18 (the extra /16 feeds the V = w/16 state
    encoding). The scan is ONE giant fused LIF+count DVE op per (t, n-half)
    on [128 x 4096]: state U = V + count/2 in a single fp32 (see
    _get_lifcnt_op). A final flush step (a = 0) counts the last spike;
    the host reads the count as rint(2U)."""
    lifcnt = _get_lifcnt_op()
    nc = bacc.Bacc("TRN2", target_bir_lowering=False)
    hh_d = nc.dram_tensor("hTh", [128, KC, TN], F16, kind="ExternalInput")
    hb_d = nc.dram_tensor("hB8", [128, KC, TN], F8E4, kind="ExternalInput")
    hc_d = nc.dram_tensor("hC8", [128, KC, TN], F8E4, kind="ExternalInput")
    ws_d = nc.dram_tensor("wS", [128, KC, VSH], F16, kind="ExternalInput")
    wb_d = nc.dram_tensor("wB8", [128, KC, VSH], F8E4, kind="ExternalInput")
    wc_d = nc.dram_tensor("wC8", [128, KC, VSH], F8E4, kind="ExternalInput")
    o_d = nc.dram_tensor("out_nh", [2, 128, VSH], F32, kind="ExternalOutput")

    VB = VSH // 512  # 8 psum-bank columns
    with tile.TileContext(nc) as tc:
        with tc.tile_pool(name="sb", bufs=1) as sb, \
             tc.tile_pool(name="ps", bufs=1, space="PSUM") as ps:

            hh = sb.tile([128, KC, TN], F16)
            hb = sb.tile([128, KC, TN], F8E4)
            hc = sb.tile([128, KC, TN], F8E4)
            ws = sb.tile([128, KC, VSH], F16)
            wb = sb.tile([128, KC, VSH], F8E4)
            wc = sb.tile([128, KC, VSH], F8E4)
            # interleave DMAs so tile-0 operands land first
            QT, QV = TN // 4, VSH // 4
            for q in range(4):
                for kc in range(KC):
                    nc.sync.dma_start(ws[:, kc, q * QV:(q + 1) * QV],
                                      ws_d.ap()[:, kc, q * QV:(q + 1) * QV])
                    nc.sync.dma_start(hh[:, kc, q * QT:(q + 1) * QT],
                                      hh_d.ap()[:, kc, q * QT:(q + 1) * QT])
                    nc.sync.dma_start(wb[:, kc, q * QV:(q + 1) * QV],
                                      wb_d.ap()[:, kc, q * QV:(q + 1) * QV])
                    nc.sync.dma_start(wc[:, kc, q * QV:(q + 1) * QV],
                                      wc_d.ap()[:, kc, q * QV:(q + 1) * QV])
                    nc.sync.dma_start(hb[:, kc, q * QT:(q + 1) * QT],
                                      hb_d.ap()[:, kc, q * QT:(q + 1) * QT])
                    nc.sync.dma_start(hc[:, kc, q * QT:(q + 1) * QT],
                                      hc_d.ap()[:, kc, q * QT:(q + 1) * QT])

            w_st = [sb.tile([128, 32, 128], F32, name=f"wst{nh}")
                    for nh in range(2)]
            zeros = sb.tile([128, 32, 128], F32, name="zeros")
            for nh in range(2):
                nc.vector.memset(w_st[nh][:], 0.0)
            nc.vector.memset(zeros[:], 0.0)

            a_ring = [sb.tile([128, 32, 128], F32, name=f"a{k}")
                      for k in range(4)]

            for t in range(T):
                for nh in range(2):
                    tb = t * 2 + nh
                    slot = a_ring[tb % 4]
                    for vh in range(2):
                        bank = ps.tile([128, 2048], F32, tag="mm",
                                       name=f"mm{tb}_{vh}", bufs=2)
                        hsl = slice(tb * 128, (tb + 1) * 128)
                        # A: fp16 hi passes (pass-outer, bank-inner: one
                        # weight load streams 4 x 512)
                        for kc in range(KC):
                            lhsT = hh[:, kc, hsl]
                            for b in range(4):
                                off = vh * 2048 + b * 512
                                nc.tensor.matmul(
                                    bank[:, b * 512:(b + 1) * 512], lhsT,
                                    ws[:, kc, off:off + 512],
                                    start=(kc == 0), stop=False)
                        # B, C: fp8 DoubleRow, K=256 in one pass each
                        for i, (hsrc, wsrc) in enumerate(((hb, wb), (hc, wc))):
                            lhsT = hsrc[:, :, hsl]
                            for b in range(4):
                                off = vh * 2048 + b * 512
                                nc.tensor.matmul(
                                    bank[:, b * 512:(b + 1) * 512], lhsT,
                                    wsrc[:, :, off:off + 512],
                                    start=False, stop=(i == 1 and b == 3),
                                    perf_mode=DR)
                        nc.scalar.activation(
                            slot[:, vh * 16:(vh + 1) * 16, :], bank[:],
                            AF.Identity, bias=0.0, scale=2.0 ** -18)
                    # fused LIF + count scan step, in-place state
                    nc.vector._custom_dve(
                        lifcnt, out=w_st[nh][:], in0=w_st[nh][:],
                        in1=slot[:], s0=0.25, s1=1.5 * 2.0 ** 22)

            for nh in range(2):
                # flush: one extra step (a=0) counts the final state's spike
                nc.vector._custom_dve(
                    lifcnt, out=w_st[nh][:], in0=w_st[nh][:],
                    in1=zeros[:], s0=0.25, s1=1.5 * 2.0 ** 22)
                nc.sync.dma_start(o_d.ap()[nh], w_st[nh][:])
    nc.compile()
    return nc


_CACHE = {}
TRACE = False
LAST = {}


def _run(nc, in_maps, key):
    import tempfile

    if TRACE:
        td = tempfile.mkdtemp(prefix=f"bkt_{key}_")
        res = run_bass_kernel_spmd(nc, in_maps, core_ids=list(range(NCORE)),
                                   trace=True, tmpdir=td)
        LAST[key] = (res, td)
        return res
    return run_bass_kernel_spmd(nc, in_maps, core_ids=list(range(NCORE)))


def _get_programs(ln_id):
    key = f"blocks{ln_id}"
    if key not in _CACHE:
        _CACHE[key] = build_blocks(ln_id=ln_id)
    if "head" not in _CACHE:
        _CACHE["head"] = build_head()
    return _CACHE[key], _CACHE["head"]


def _pack_weights(Wr, Wk, Wv, Wo, W1, b1, W2, b2, g1, be1, g2, be2):
    import ml_dtypes
    e4t = ml_dtypes.float8_e4m3
    w16 = np.zeros((L, 128, W16), np.float16)
    w8 = np.zeros((L, 128, 2, NB8, 2, 128), e4t)
    w32 = np.zeros((L, 128, WS), np.float32)
    for l in range(L):
        his = []

        def add(mat):  # [K, M] fp32 -> fp16 hi (+ lo for Wo/W2)
            hi, lo = _split16(mat)
            his.append(hi)
            return lo

        # gates + W1: hi fp16 pre-scaled 2^14; B/C corrections fp8 DR tiles
        gh = []
        for bank in range(NB8):
            for kc in range(KC):
                if bank < 6:
                    g, hf = divmod(bank, KC)
                    Wg = (Wr, Wk, Wv)[g]
                    blk = 0.5 * Wg[l][kc * 128:(kc + 1) * 128,
                                      hf * 128:(hf + 1) * 128]
                else:
                    mf = bank - 6
                    blk = 0.5 * W1[l][kc * 128:(kc + 1) * 128,
                                      mf * 128:(mf + 1) * 128]
                hi = blk.astype(np.float16)
                lo = blk - hi.astype(np.float32)
                gh.append((hi.astype(np.float32) * 2.0 ** 14)
                          .astype(np.float16))
                w8[l, :, 0, bank, kc, :] = (hi.astype(np.float32) * 2.0 ** 3
                                            ).astype(e4t)
                w8[l, :, 1, bank, kc, :] = (lo * 2.0 ** 14).astype(e4t)
        ghs = np.concatenate(gh[:12], axis=1)     # gates hi
        w1h = np.concatenate(gh[12:], axis=1)     # W1 hi
        his = []
        los = []
        for hf in range(KC):
            for kc in range(KC):
                los.append(add(Wo[l][kc * 128:(kc + 1) * 128,
                                     hf * 128:(hf + 1) * 128]))
        woh = np.concatenate(his, axis=1)
        wol = np.concatenate([x.astype(np.float16) for x in los], axis=1)
        his, los = [], []
        for mh in range(KC):
            for kc8 in range(FC):
                los.append(add(0.5 * W2[l][kc8 * 128:(kc8 + 1) * 128,
                                           mh * 128:(mh + 1) * 128]))
        w2h = np.concatenate(his, axis=1)
        w2l = np.concatenate([x.astype(np.float16) for x in los], axis=1)
        w16[l] = np.concatenate([ghs, woh, wol, w1h, w2h, w2l], axis=1)
        w32[l] = np.concatenate([
            0.5 * b1[l].reshape(FC, 128).T,
            0.5 * b2[l].reshape(KC, 128).T,
            g1[l].reshape(KC, 128).T, be1[l].reshape(KC, 128).T,
            g2[l].reshape(KC, 128).T, be2[l].reshape(KC, 128).T,
        ], axis=1)
    return (np.ascontiguousarray(w16), np.ascontiguousarray(w8),
            np.ascontiguousarray(w32))


def kernel(input_ids, token_embedding, pos_embedding, noise, unif,
           Wr, Wk, Wv, Wo, W1, b1, W2, b2, ln1_g, ln1_b, ln2_g, ln2_b,
           Wout, bout):
    input_ids = np.asarray(input_ids)
    f32 = lambda a: np.asarray(a, dtype=np.float32)
    token_embedding, pos_embedding, noise, unif = map(
        f32, (token_embedding, pos_embedding, noise, unif))
    Wr, Wk, Wv, Wo, W1, b1, W2, b2 = map(f32, (Wr, Wk, Wv, Wo, W1, b1, W2, b2))
    ln1_g, ln1_b, ln2_g, ln2_b, Wout, bout = map(
        f32, (ln1_g, ln1_b, ln2_g, ln2_b, Wout, bout))

    ln_id = bool((ln1_g == 1).all() and (ln1_b == 0).all()
                 and (ln2_g == 1).all() and (ln2_b == 0).all())
    nc_blocks, nc_head = _get_programs(ln_id)

    spikes = _encode_spikes(input_ids, token_embedding, pos_embedding, noise, unif)
    sp = spikes.reshape(T, NCORE, R, KC, 128)          # (t, core, r, kc, p)
    x0 = np.ascontiguousarray(sp.transpose(1, 4, 3, 0, 2)).reshape(NCORE, 128, KC, TR)
    w16, w8, w32 = _pack_weights(Wr, Wk, Wv, Wo, W1, b1, W2, b2,
                                 ln1_g, ln1_b, ln2_g, ln2_b)
    in1 = [{"x0": x0[c], "w16": w16, "w8": w8, "w32": w32}
           for c in range(NCORE)]
    res1 = _run(nc_blocks, in1, "blocks")
    ho = np.stack([res1.results[c]["h_out"].reshape(128, KC, T, R)
                   for c in range(NCORE)])
    hT = np.ascontiguousarray(ho.transpose(1, 2, 3, 0, 4)).reshape(128, KC, TN)
    import ml_dtypes
    e4 = ml_dtypes.float8_e4m3
    hTh16 = hT.astype(np.float16)
    hTl = hT - hTh16.astype(np.float32)
    hB8 = np.ascontiguousarray((hTl * 2.0 ** 11).astype(e4))
    hC8 = np.ascontiguousarray(hTh16.astype(np.float32).astype(e4))
    hTh16 = np.ascontiguousarray(hTh16)

    assert not np.any(bout), "head kernel assumes bout == 0 (spec fill=zeros)"
    Wp = np.zeros((D, VPAD), np.float32)
    Wp[:, :V] = 0.5 * Wout
    Wph16 = Wp.astype(np.float16)
    Wpl = Wp - Wph16.astype(np.float32)
    WSc = (Wph16.astype(np.float32) * 2.0 ** 14).astype(np.float16)
    WB8 = (Wph16.astype(np.float32) * 2.0 ** 3).astype(e4)
    WC8 = (Wpl * 2.0 ** 14).astype(e4)

    def shard(Wx, c):
        w = Wx[:, c * VSH:(c + 1) * VSH].reshape(KC, 128, VSH)
        return np.ascontiguousarray(w.transpose(1, 0, 2))
    in2 = [{"hTh": hTh16, "hB8": hB8, "hC8": hC8,
            "wS": shard(WSc, c), "wB8": shard(WB8, c), "wC8": shard(WC8, c)}
           for c in range(NCORE)]
    res2 = _run(nc_head, in2, "head")
    # out_nh[nh, p, v] holds U = V + count/2: count = rint(2U).
    # row n = nh*128 + p, vocab col = c*VSH + v
    out_sh = np.stack([res2.results[c]["out_nh"] for c in range(NCORE)])
    out = np.empty((N, VPAD), np.float32)
    for c in range(NCORE):
        for nh in range(2):
            out[nh * 128:(nh + 1) * 128, c * VSH:(c + 1) * VSH] = \
                np.rint(2.0 * out_sh[c, nh].astype(np.float64)).reshape(128, VSH)
    out = out[:, :V].reshape(B, S, V).astype(np.float32)
    return out

